# revision 1
# baseline (speedup 1.0000x reference)
"""CategorySpecificLinear Trainium2 kernel.

out[t] = x[t] @ weight[category_id[t]] + bias[category_id[t]]

Strategy: expert-parallel over the 8 categories (C == n_cores == 8).
On the host we route tokens by category (the "all-to-all" happens at
sharding time since we receive full inputs), transpose each category's
token block to [D, T_pad] (the PE needs the contraction dim on
partitions and fp32 has no DMA-transpose), and hand core c:
    xT   [D, T_pad]   tokens of category c, zero-padded to T_pad
    w    [D, O]       weight[c]
    bias [1, O]       bias[c]
Each core computes out = xT.T @ w + bias with fp32r matmuls (full fp32
precision at 1 col/cycle for N>=256), then the host scatters the rows
back to token order.

Per-core HBM traffic ~8.7 MB (x 2.2 + w 4 + bias-bcast 0.5 + out 2.2);
fp32r MMs measure ~390 ns warm at N=512, so the 80-matmul stream is
~22 us and overlaps the ~24 us DMA stream. Measured on HW: 43.3 us
NEFF exec (incl. ~17 us fixed framework preamble/tail), rel err 1.5e-4.
"""

import contextlib
import ctypes
import os
import sys
import types

import numpy as np

sys.path.insert(0, "/opt/trn_rl_repo")


def _ensure_ntff_hook():
    """Provide antenv.axon_hooks if the image lacks it.

    concourse.bass_utils imports antenv.axon_hooks.get_axon_ntff_profile_hook
    when trace=True under axon; some agent images don't ship that module, in
    which case the boot's NTFF hook registration silently degrades and the
    import in bass_utils crashes. Recreate the slim ctypes hook here
    (mirrors trn_agent_boot.trn_boot._ntff_profile_via_ctypes).
    """
    try:
        import antenv.axon_hooks  # noqa: F401

        return
    except ImportError:
        pass

    so_path = "/opt/axon/libaxon_pjrt.so"
    hook = None
    if os.path.exists(so_path):
        lib = ctypes.CDLL(so_path)
        if hasattr(lib, "axon_start_nrt_profile"):
            lib.axon_start_nrt_profile.argtypes = [
                ctypes.POINTER(ctypes.c_int64),
                ctypes.c_size_t,
            ]
            lib.axon_start_nrt_profile.restype = ctypes.c_int64
            lib.axon_stop_nrt_profile.argtypes = [ctypes.c_char_p]
            lib.axon_stop_nrt_profile.restype = ctypes.c_int64

            @contextlib.contextmanager
            def hook(output_dir, device_ids):
                import jax

                jax.devices()
                if device_ids:
                    ids = (ctypes.c_int64 * len(device_ids))(*device_ids)
                    rc = lib.axon_start_nrt_profile(ids, len(device_ids))
                else:
                    rc = lib.axon_start_nrt_profile(None, 0)
                if rc != 0:
                    raise RuntimeError(f"axon_start_nrt_profile rc={rc}")
                try:
                    yield
                finally:
                    n = lib.axon_stop_nrt_profile(str(output_dir).encode())
                    if n <= 0:
                        print(
                            f"ntff profile: rc={n} writing {output_dir}",
                            file=sys.stderr,
                        )

    mod = types.ModuleType("antenv.axon_hooks")
    _state = {"hook": hook}
    mod.set_axon_ntff_profile_hook = lambda h: _state.__setitem__("hook", h)
    mod.get_axon_ntff_profile_hook = lambda: _state["hook"]
    sys.modules["antenv.axon_hooks"] = mod
    try:
        import antenv

        antenv.axon_hooks = mod
    except ImportError:
        pass


_ensure_ntff_hook()

import concourse.bass as bass
import concourse.bacc as bacc_mod
import concourse.mybir as mybir
import concourse.tile as tile
from concourse.bass import ts
from concourse.bass_utils import run_bass_kernel_spmd

N_CORES = 8
P = 128
N_TILE = 512  # one fp32 PSUM bank; also >=256 keeps fp32r at full rate

_nc_cache = {}
LAST_RESULTS = None  # BassKernelResults of the most recent run (for test.py)


def _build_nc(T_pad: int, D: int, O: int):
    KO = D // P
    NO = O // N_TILE
    mmdt = mybir.dt.float32r
    f32 = mybir.dt.float32

    # m-tiles: full 128-row tiles plus one remainder tile (multiple of 32)
    m_sizes = [P] * (T_pad // P)
    if T_pad % P:
        m_sizes.append(T_pad % P)
    MO = len(m_sizes)
    m_starts = [sum(m_sizes[:i]) for i in range(MO)]

    nc = bacc_mod.Bacc()
    xT = nc.dram_tensor("xT", [D, T_pad], mmdt, kind="ExternalInput")
    w = nc.dram_tensor("w", [D, O], mmdt, kind="ExternalInput")
    bias = nc.dram_tensor("bias", [1, O], f32, kind="ExternalInput")
    out = nc.dram_tensor("out", [T_pad, O], f32, kind="ExternalOutput")

    xT_t = xT[:, :].rearrange("(ko p) t -> p ko t", p=P)
    w_t = w[:, :].rearrange("(ko p) o -> p ko o", p=P)

    with tile.TileContext(nc) as tc:
        with (
            tc.tile_pool(name="resident", bufs=1) as rpool,
            tc.tile_pool(name="psum", bufs=7, space="PSUM") as psum_pool,
            tc.tile_pool(name="warmps", bufs=1, space="PSUM") as warm_pool,
            tc.tile_pool(name="obuf", bufs=6) as opool,
        ):
            # HAM warm-up: the PE is otherwise idle until the first k-slice
            # lands (~11 us); ~5 us of dummy matmuls gets the clock gate to
            # 8/8 so the real fp32r stream starts at warm speed (389 ns vs
            # 628 ns per MM). The dummy psum bank is never read.
            warm_sb = rpool.tile([P, 64], f32, tag="warm")
            nc.vector.memset(warm_sb[:], 0.0)
            warm_ps = warm_pool.tile([64, 64], f32, tag="wps")
            for i in range(24):
                nc.tensor.matmul(
                    warm_ps[:],
                    lhsT=warm_sb[:, :64],
                    rhs=warm_sb[:, :64],
                    start=True,
                    stop=True,
                )
            # Loads split over the two HWDGE engines (~650 ns serialized
            # issue cost each; one ~200 GB/s queue per engine). k-major so
            # wave A starts after the first k-slice pair, not the full 6 MB.
            # The 512 KB bias broadcast queues behind w(0..1,0) so it does
            # not delay the first matmul (DVE needs it much later).
            bias_sb = rpool.tile([P, O], f32, tag="bias")
            x_sb = []
            w_sb = {}
            for k in range(KO):
                xt = rpool.tile([P, T_pad], mmdt, tag=f"x{k}")
                nc.sync.dma_start(xt[:], xT_t[:, k, :])
                x_sb.append(xt)
                wt = rpool.tile([P, N_TILE], mmdt, tag=f"w{k}_0")
                nc.scalar.dma_start(wt[:], w_t[:, k, ts(0, N_TILE)])
                w_sb[(k, 0)] = wt
                if k == 1:
                    # broadcast from DRAM on the idle GpSimd queue so the
                    # 512 KB doesn't delay the w(:,0) stream on ACT
                    nc.gpsimd.dma_start(
                        bias_sb[:], bias[:, :].to_broadcast((P, O))
                    )
            for k in range(KO):
                for n in range(1, NO):
                    wt = rpool.tile([P, N_TILE], mmdt, tag=f"w{k}_{n}")
                    eng = nc.sync if k % 2 == 0 else nc.scalar
                    eng.dma_start(wt[:], w_t[:, k, ts(n, N_TILE)])
                    w_sb[(k, n)] = wt

            # One wave per n-tile: all MO psum groups accumulate in lockstep
            # over k, so the k-th step only needs x(k)/w(k,n) — PE starts
            # after the first ~600 KB instead of after the full 6 MB.
            for n in range(NO):
                pss = [
                    psum_pool.tile([m_sizes[m], N_TILE], f32, tag="ps", name=f"ps{n}_{m}")
                    for m in range(MO)
                ]
                for k in range(KO):
                    for m in range(MO):
                        nc.tensor.matmul(
                            pss[m][:],
                            lhsT=x_sb[k][:, m_starts[m] : m_starts[m] + m_sizes[m]],
                            rhs=w_sb[(k, n)][:],
                            start=(k == 0),
                            stop=(k == KO - 1),
                        )
                for m in range(MO):
                    ot = opool.tile([P, N_TILE], f32, tag="ot", name=f"ot{n}_{m}")
                    nc.vector.tensor_add(
                        ot[: m_sizes[m]], pss[m][:], bias_sb[: m_sizes[m], ts(n, N_TILE)]
                    )
                    nc.gpsimd.dma_start(
                        out[m_starts[m] : m_starts[m] + m_sizes[m], ts(n, N_TILE)],
                        ot[: m_sizes[m]],
                    )
    nc.finalize()
    return nc


def kernel(x, category_id, weight, bias):
    global LAST_RESULTS
    x = np.asarray(x)
    category_id = np.asarray(category_id)
    weight = np.ascontiguousarray(np.asarray(weight), dtype=np.float32)
    bias = np.ascontiguousarray(np.asarray(bias), dtype=np.float32)

    orig_shape = x.shape
    D = orig_shape[-1]
    C, _, O = weight.shape
    assert C == N_CORES and D % P == 0 and O % N_TILE == 0

    T = int(np.prod(orig_shape[:-1]))
    x_flat = np.ascontiguousarray(x.reshape(T, D), dtype=np.float32)
    cid = category_id.reshape(T).astype(np.int64)

    idx_per_c = [np.flatnonzero(cid == c) for c in range(C)]
    counts = [len(ix) for ix in idx_per_c]
    T_pad = max(32, -(-max(counts) // 32) * 32)  # multiple of 32 (PE col-group)

    key = (T_pad, D, O)
    if key not in _nc_cache:
        _nc_cache[key] = _build_nc(T_pad, D, O)
    nc = _nc_cache[key]

    in_maps = []
    for c in range(C):
        xcT = np.zeros((D, T_pad), dtype=np.float32)
        xcT[:, : counts[c]] = x_flat[idx_per_c[c]].T
        in_maps.append(
            {
                "xT": xcT,
                "w": weight[c],
                "bias": bias[c : c + 1],
            }
        )

    res = run_bass_kernel_spmd(nc, in_maps, list(range(N_CORES)))
    LAST_RESULTS = res

    out_flat = np.empty((T, O), dtype=np.float32)
    for c in range(C):
        out_flat[idx_per_c[c]] = res.results[c]["out"][: counts[c]]
    return out_flat.reshape(*orig_shape[:-1], O)



# revision 4
# speedup vs baseline: 1.1709x; 1.1709x over previous
"""CategorySpecificLinear Trainium2 kernel (v2: bf16 weight-stationary).

out[t] = x[t] @ weight[category_id[t]] + bias[category_id[t]]

Strategy: expert-parallel over the 8 categories (C == n_cores == 8).
Host routes tokens by category; core c computes its category's tokens.

Device-side formulation (per core, transposed output):
    outT[o, t] = sum_k wT[k, o] * xT[k, t] + bias[o]
with the weight tile [128k x 128o] STATIONARY in the PE array and x
streamed as the moving operand, so PE stream cycles = OT*KO*T_pad
(34k cycles ~ 14.1 us warm @2.4GHz) with zero m-tile quantization
waste. All operands bf16 (psum accumulates fp32), which halves HBM
traffic vs fp32 and runs the PE at 1 col/cycle warm.

Loop structure: two phases of 4 o-tiles each, k-OUTER inside a phase:
    for k: for ot in phase: MM(psA[ot], w[k,ot], x[k][:,:TA])
                            MM(psB[ot], w[k,ot], x[k][:,TA:])
so the PE starts as soon as the first (x k-slice, w k-slice) chunk
lands (~1.5 us) instead of after the full 3 MB load, and each phase's
outputs (DVE bias-add + bf16 store) drain while the next phase
computes. 8 psum banks = 4 o-tiles x 2 T-halves in flight.

DMA: host pre-arranges x as [128p, ko, t] and w as [128p, ko, ot, o]
(k-major per partition) so chunked loads are plain contiguous 2D
slices: 4 x-chunks on the sync HWDGE queue, 4 w-chunks on the scalar
HWDGE queue (~620 ns issue each), bias on the idle gpsimd queue.
Per-core HBM: x 1.06 MB + w 2 MB + out 1.06 MB bf16 = 4.2 MB (~12 us
wire), fully overlapped with the ~15 us PE stream.

bf16 numerics: rel err ~1e-3 on dot-1024 (gate is 2e-2).
"""

import contextlib
import ctypes
import os
import sys
import types

import numpy as np

sys.path.insert(0, "/opt/trn_rl_repo")


def _ensure_ntff_hook():
    """Provide antenv.axon_hooks if the image lacks it.

    concourse.bass_utils imports antenv.axon_hooks.get_axon_ntff_profile_hook
    when trace=True under axon; some agent images don't ship that module, in
    which case the boot's NTFF hook registration silently degrades and the
    import in bass_utils crashes. Recreate the slim ctypes hook here
    (mirrors trn_agent_boot.trn_boot._ntff_profile_via_ctypes).
    """
    try:
        import antenv.axon_hooks  # noqa: F401

        return
    except ImportError:
        pass

    so_path = "/opt/axon/libaxon_pjrt.so"
    hook = None
    if os.path.exists(so_path):
        lib = ctypes.CDLL(so_path)
        if hasattr(lib, "axon_start_nrt_profile"):
            lib.axon_start_nrt_profile.argtypes = [
                ctypes.POINTER(ctypes.c_int64),
                ctypes.c_size_t,
            ]
            lib.axon_start_nrt_profile.restype = ctypes.c_int64
            lib.axon_stop_nrt_profile.argtypes = [ctypes.c_char_p]
            lib.axon_stop_nrt_profile.restype = ctypes.c_int64

            @contextlib.contextmanager
            def hook(output_dir, device_ids):
                import jax

                jax.devices()
                if device_ids:
                    ids = (ctypes.c_int64 * len(device_ids))(*device_ids)
                    rc = lib.axon_start_nrt_profile(ids, len(device_ids))
                else:
                    rc = lib.axon_start_nrt_profile(None, 0)
                if rc != 0:
                    raise RuntimeError(f"axon_start_nrt_profile rc={rc}")
                try:
                    yield
                finally:
                    n = lib.axon_stop_nrt_profile(str(output_dir).encode())
                    if n <= 0:
                        print(
                            f"ntff profile: rc={n} writing {output_dir}",
                            file=sys.stderr,
                        )

    mod = types.ModuleType("antenv.axon_hooks")
    _state = {"hook": hook}
    mod.set_axon_ntff_profile_hook = lambda h: _state.__setitem__("hook", h)
    mod.get_axon_ntff_profile_hook = lambda: _state["hook"]
    sys.modules["antenv.axon_hooks"] = mod
    try:
        import antenv

        antenv.axon_hooks = mod
    except ImportError:
        pass


_ensure_ntff_hook()

import ml_dtypes

import concourse.bass as bass
import concourse.bacc as bacc_mod
import concourse.mybir as mybir
import concourse.tile as tile
from concourse.bass import ts
from concourse.bass_utils import run_bass_kernel_spmd

N_CORES = 8
P = 128
BF16 = np.dtype(ml_dtypes.bfloat16)

_nc_cache = {}
LAST_RESULTS = None  # BassKernelResults of the most recent run (for test.py)

# k-chunk sizes for the x and w loads (small first chunks so the PE can
# start after ~400 KB instead of ~1 MB)
XCH = [1, 2, 2, 3]
WCH = [1, 1, 3, 3]


def _cum(sizes):
    out = [0]
    for s in sizes:
        out.append(out[-1] + s)
    return out


def _build_nc(T_pad: int, D: int, O: int):
    KO = D // P
    OT = O // P
    assert KO == 8 and OT == 8
    bf = mybir.dt.bfloat16
    f32 = mybir.dt.float32

    # moving-operand split: psum bank holds <=512 fp32, so stream T in
    # two halves (both multiples of 16)
    if T_pad <= 512:
        TA, TB = T_pad, 0
    else:
        TA = -(-(T_pad // 2) // 16) * 16
        TB = T_pad - TA
        assert TB <= 512

    cx = _cum(XCH)
    cw = _cum(WCH)

    nc = bacc_mod.Bacc()
    x = nc.dram_tensor("x", [P, KO * T_pad], bf, kind="ExternalInput")
    w = nc.dram_tensor("w", [P, KO * OT * P], bf, kind="ExternalInput")
    bias = nc.dram_tensor("bias", [P, OT], f32, kind="ExternalInput")
    out = nc.dram_tensor("out", [O, T_pad], bf, kind="ExternalOutput")

    with tile.TileContext(nc) as tc:
        with (
            tc.tile_pool(name="resident", bufs=1) as rpool,
            tc.tile_pool(name="psum", bufs=1, space="PSUM") as psum_pool,
            tc.tile_pool(name="obuf", bufs=4) as opool,
        ):
            # ---- input DMAs -------------------------------------------------
            bias_sb = rpool.tile([P, OT], f32, tag="bias")
            nc.gpsimd.dma_start(bias_sb[:], bias[:, :])

            x_sb = []
            for j, kn in enumerate(XCH):
                xt = rpool.tile([P, kn * T_pad], bf, tag=f"x{j}")
                nc.sync.dma_start(xt[:], x[:, cx[j] * T_pad : (cx[j] + kn) * T_pad])
                x_sb.append(xt)
            w_sb = []
            for j, kn in enumerate(WCH):
                wt = rpool.tile([P, kn * O], bf, tag=f"w{j}")
                nc.scalar.dma_start(wt[:], w[:, cw[j] * O : (cw[j] + kn) * O])
                w_sb.append(wt)

            def xs(k):
                j = next(i for i in range(len(XCH)) if cx[i] <= k < cx[i + 1])
                return x_sb[j], (k - cx[j]) * T_pad

            def wsl(k, ot):
                j = next(i for i in range(len(WCH)) if cw[i] <= k < cw[i + 1])
                base = (k - cw[j]) * O + ot * P
                return w_sb[j][:, base : base + P]

            # ---- PE warm-up -------------------------------------------------
            # HAM un-throttles after ~3.4us of sustained PE activity; dummy
            # matmuls during the initial DMA wait start that clock early.
            # Results land in psA0 which the first real MM (start=True)
            # overwrites.
            warm_sb = rpool.tile([P, P], bf, tag="warm")
            nc.vector.memset(warm_sb[:], 0.0)

            NPH = 2
            OPH = OT // NPH
            for ph in range(NPH):
                psA = [
                    psum_pool.tile([P, TA], f32, tag=f"psA{i}", name=f"psA{ph}_{i}")
                    for i in range(OPH)
                ]
                psB = [
                    psum_pool.tile([P, TB], f32, tag=f"psB{i}", name=f"psB{ph}_{i}")
                    for i in range(OPH)
                ] if TB else None

                if ph == 0:
                    for _ in range(14):
                        nc.tensor.matmul(
                            psA[0][:, :P],
                            lhsT=warm_sb[:],
                            rhs=warm_sb[:],
                            start=True,
                            stop=True,
                        )

                for k in range(KO):
                    for i in range(OPH):
                        ot = ph * OPH + i
                        lhsT = wsl(k, ot)
                        xt, base = xs(k)
                        nc.tensor.matmul(
                            psA[i][:],
                            lhsT=lhsT,
                            rhs=xt[:, base : base + TA],
                            start=(k == 0),
                            stop=(k == KO - 1),
                        )
                        if TB:
                            nc.tensor.matmul(
                                psB[i][:],
                                lhsT=lhsT,
                                rhs=xt[:, base + TA : base + T_pad],
                                start=(k == 0),
                                stop=(k == KO - 1),
                            )

                for i in range(OPH):
                    ot = ph * OPH + i
                    o_sb = opool.tile([P, T_pad], bf, tag="ot", name=f"o{ph}_{i}")
                    bcol = bias_sb[:, ot : ot + 1]
                    nc.vector.tensor_scalar_add(o_sb[:, :TA], psA[i][:], bcol)
                    if TB:
                        nc.vector.tensor_scalar_add(o_sb[:, TA:], psB[i][:], bcol)
                    eng = nc.sync if ot % 2 == 0 else nc.scalar
                    eng.dma_start(out[ot * P : (ot + 1) * P, :], o_sb[:])
    nc.finalize()
    return nc


def kernel(x, category_id, weight, bias):
    global LAST_RESULTS
    x = np.asarray(x)
    category_id = np.asarray(category_id)
    weight = np.ascontiguousarray(np.asarray(weight), dtype=np.float32)
    bias = np.ascontiguousarray(np.asarray(bias), dtype=np.float32)

    orig_shape = x.shape
    D = orig_shape[-1]
    C, _, O = weight.shape
    KO, OT = D // P, O // P
    assert C == N_CORES and KO == 8 and OT == 8

    T = int(np.prod(orig_shape[:-1]))
    x_flat = np.ascontiguousarray(x.reshape(T, D), dtype=np.float32)
    cid = category_id.reshape(T).astype(np.int64)

    idx_per_c = [np.flatnonzero(cid == c) for c in range(C)]
    counts = [len(ix) for ix in idx_per_c]
    T_pad = max(32, -(-max(counts) // 16) * 16)

    key = (T_pad, D, O)
    if key not in _nc_cache:
        _nc_cache[key] = _build_nc(T_pad, D, O)
    nc = _nc_cache[key]

    # pre-arranged per-partition-contiguous layouts (see _build_nc)
    in_maps = []
    for c in range(C):
        xc = np.zeros((T_pad, D), dtype=np.float32)
        xc[: counts[c]] = x_flat[idx_per_c[c]]
        # [t, (k p)] -> [p, k, t]
        xh = np.ascontiguousarray(
            xc.T.reshape(KO, P, T_pad).transpose(1, 0, 2), dtype=np.float32
        ).astype(BF16).reshape(P, KO * T_pad)
        # [(k p), (ot o)] -> [p, k, ot, o]
        wh = np.ascontiguousarray(
            weight[c].reshape(KO, P, OT, P).transpose(1, 0, 2, 3),
            dtype=np.float32,
        ).astype(BF16).reshape(P, KO * O)
        # [ (ot o) ] -> [o, ot]
        bh = np.ascontiguousarray(bias[c].reshape(OT, P).T)
        in_maps.append({"x": xh, "w": wh, "bias": bh})

    res = run_bass_kernel_spmd(nc, in_maps, list(range(N_CORES)))
    LAST_RESULTS = res

    out_flat = np.empty((T, O), dtype=np.float32)
    for c in range(C):
        oc = np.asarray(res.results[c]["out"])  # [O, T_pad] bf16
        out_flat[idx_per_c[c]] = oc[:, : counts[c]].T.astype(np.float32)
    return out_flat.reshape(*orig_shape[:-1], O)


# revision 5
# speedup vs baseline: 1.1971x; 1.0223x over previous
"""CategorySpecificLinear Trainium2 kernel (v3: bf16 weight-stationary).

out[t] = x[t] @ weight[category_id[t]] + bias[category_id[t]]

Strategy: expert-parallel over the 8 categories (C == n_cores == 8).
Host routes tokens by category; core c computes its category's tokens.

Device-side formulation (per core, transposed output):
    outT[o, t] = sum_k wT[k, o] * xT[k, t] + bias[o]
with the weight tile [128k x 128o] STATIONARY in the PE array and x
streamed as the moving operand, so PE stream cycles = OT*KO*T_pad
(~34k cycles ~ 14.1 us warm @2.4GHz) with zero m-tile quantization
waste. All operands bf16 (psum accumulates fp32) -> half the HBM
traffic of fp32 and 1 col/cycle warm on the PE.

Schedule (8 psum banks = 4 o-tiles x 2 T-halves in flight):
  phase 0 (o-tiles 0-3): k-OUTER -- each k-step needs only x[k] and
    w[g0,k], so the PE starts ~1.5 us after the first small chunks
    land instead of after the full 3 MB input load.
  phase 1 (o-tiles 4-7): everything is SBUF-resident by now, so run
    ot-OUTER: each o-tile's output drains (DVE/ACT bias-add + bf16
    store) while the next o-tile computes -> only the last o-tile's
    add+store is kernel tail.
Warm-up: ~24 dummy matmuls bridge the initial DMA wait so the HAM
clock gate reaches 8/8 before the real stream.

DMA: three queues (sync + scalar HWDGE, gpsimd SWDGE) each see
~190 GB/s when all active and lose ~1 us between chained DMAs, so
inputs are split in 2 chunks per queue (small first chunk for early
PE start): sync = x, scalar = w[o-tiles 0-3], gpsimd = bias + w[o-
tiles 4-7]. Host pre-arranges x as [p][k][t] and w as [p][g][k][o]
(per-partition contiguous) so every load is a plain 2D slice.
Per-core HBM: x 1.06 + w 2 + out 1.06 MB = 4.2 MB.

bf16 numerics: rel err ~3e-3 on dot-1024 (gate is 2e-2).
"""

import contextlib
import ctypes
import os
import sys
import types

import numpy as np

sys.path.insert(0, "/opt/trn_rl_repo")


def _ensure_ntff_hook():
    """Provide antenv.axon_hooks if the image lacks it.

    concourse.bass_utils imports antenv.axon_hooks.get_axon_ntff_profile_hook
    when trace=True under axon; some agent images don't ship that module, in
    which case the boot's NTFF hook registration silently degrades and the
    import in bass_utils crashes. Recreate the slim ctypes hook here
    (mirrors trn_agent_boot.trn_boot._ntff_profile_via_ctypes).
    """
    try:
        import antenv.axon_hooks  # noqa: F401

        return
    except ImportError:
        pass

    so_path = "/opt/axon/libaxon_pjrt.so"
    hook = None
    if os.path.exists(so_path):
        lib = ctypes.CDLL(so_path)
        if hasattr(lib, "axon_start_nrt_profile"):
            lib.axon_start_nrt_profile.argtypes = [
                ctypes.POINTER(ctypes.c_int64),
                ctypes.c_size_t,
            ]
            lib.axon_start_nrt_profile.restype = ctypes.c_int64
            lib.axon_stop_nrt_profile.argtypes = [ctypes.c_char_p]
            lib.axon_stop_nrt_profile.restype = ctypes.c_int64

            @contextlib.contextmanager
            def hook(output_dir, device_ids):
                import jax

                jax.devices()
                if device_ids:
                    ids = (ctypes.c_int64 * len(device_ids))(*device_ids)
                    rc = lib.axon_start_nrt_profile(ids, len(device_ids))
                else:
                    rc = lib.axon_start_nrt_profile(None, 0)
                if rc != 0:
                    raise RuntimeError(f"axon_start_nrt_profile rc={rc}")
                try:
                    yield
                finally:
                    n = lib.axon_stop_nrt_profile(str(output_dir).encode())
                    if n <= 0:
                        print(
                            f"ntff profile: rc={n} writing {output_dir}",
                            file=sys.stderr,
                        )

    mod = types.ModuleType("antenv.axon_hooks")
    _state = {"hook": hook}
    mod.set_axon_ntff_profile_hook = lambda h: _state.__setitem__("hook", h)
    mod.get_axon_ntff_profile_hook = lambda: _state["hook"]
    sys.modules["antenv.axon_hooks"] = mod
    try:
        import antenv

        antenv.axon_hooks = mod
    except ImportError:
        pass


_ensure_ntff_hook()

import ml_dtypes

import concourse.bass as bass
import concourse.bacc as bacc_mod
import concourse.mybir as mybir
import concourse.tile as tile
from concourse.bass_utils import run_bass_kernel_spmd

N_CORES = 8
P = 128
BF16 = np.dtype(ml_dtypes.bfloat16)

_nc_cache = {}
LAST_RESULTS = None  # BassKernelResults of the most recent run (for test.py)

N_WARM = 24  # dummy matmuls bridging the initial DMA wait (HAM warm-up)


def _build_nc(T_pad: int, D: int, O: int):
    KO = D // P
    OT = O // P
    assert KO == 8 and OT == 8
    bf = mybir.dt.bfloat16
    f32 = mybir.dt.float32

    # moving-operand split: one psum bank holds <=512 fp32 per partition,
    # so stream T in two halves (both multiples of 16)
    if T_pad <= 512:
        TA, TB = T_pad, 0
    else:
        TA = -(-(T_pad // 2) // 16) * 16
        TB = T_pad - TA
        assert TB <= 512

    GW = 4 * KO * P  # elems per ot-group per partition in w (4096)

    nc = bacc_mod.Bacc()
    x = nc.dram_tensor("x", [P, KO * T_pad], bf, kind="ExternalInput")
    w = nc.dram_tensor("w", [P, 2 * GW], bf, kind="ExternalInput")
    bias = nc.dram_tensor("bias", [P, OT], f32, kind="ExternalInput")
    out = nc.dram_tensor("out", [O, T_pad], bf, kind="ExternalOutput")

    # chunk boundaries (k-index splits) per queue: small first chunk so
    # the PE can start early, big second chunk to amortize the ~1us
    # inter-DMA ring gap
    XSPLIT = 2  # x: k0-1, k2-7 on sync
    WSPLIT = 2  # w g0: k0-1, k2-7 on scalar
    GSPLIT = 4  # w g1: k0-3, k4-7 on gpsimd

    with tile.TileContext(nc) as tc:
        with (
            tc.tile_pool(name="resident", bufs=1) as rpool,
            tc.tile_pool(name="psum", bufs=1, space="PSUM") as psum_pool,
            tc.tile_pool(name="obuf", bufs=4) as opool,
        ):
            # ---- input DMAs -------------------------------------------------
            x_sb = rpool.tile([P, KO * T_pad], bf, tag="x")
            nc.sync.dma_start(x_sb[:, : XSPLIT * T_pad], x[:, : XSPLIT * T_pad])
            nc.sync.dma_start(x_sb[:, XSPLIT * T_pad :], x[:, XSPLIT * T_pad :])

            w_sb = rpool.tile([P, 2 * GW], bf, tag="w")
            c1 = WSPLIT * 4 * P
            nc.scalar.dma_start(w_sb[:, :c1], w[:, :c1])
            nc.scalar.dma_start(w_sb[:, c1:GW], w[:, c1:GW])

            bias_sb = rpool.tile([P, OT], f32, tag="bias")
            nc.gpsimd.dma_start(bias_sb[:], bias[:, :])
            c2 = GW + GSPLIT * 4 * P
            nc.gpsimd.dma_start(w_sb[:, GW:c2], w[:, GW:c2])
            nc.gpsimd.dma_start(w_sb[:, c2:], w[:, c2:])

            def wsl(k, ot):
                g, i = ot // 4, ot % 4
                base = g * GW + k * 4 * P + i * P
                return w_sb[:, base : base + P]

            def xsl(k, lo, hi):
                return x_sb[:, k * T_pad + lo : k * T_pad + hi]

            # ---- PE warm-up -------------------------------------------------
            # HAM un-throttles after ~3.4us of sustained PE activity; dummy
            # matmuls during the initial DMA wait start that clock early.
            # Results land in psA0 which the first real MM (start=True)
            # overwrites.
            warm_sb = rpool.tile([P, P], bf, tag="warm")
            nc.vector.memset(warm_sb[:], 0.0)

            def emit_out(ot, psA, psB):
                o_sb = opool.tile([P, T_pad], bf, tag="ot", name=f"o{ot}")
                bcol = bias_sb[:, ot : ot + 1]
                nc.vector.tensor_scalar_add(o_sb[:, :TA], psA[:], bcol)
                if TB:
                    nc.scalar.activation(
                        o_sb[:, TA:],
                        psB[:],
                        mybir.ActivationFunctionType.Identity,
                        bias=bcol,
                        scale=1.0,
                    )
                eng = nc.sync if ot % 2 == 0 else nc.scalar
                eng.dma_start(out[ot * P : (ot + 1) * P, :], o_sb[:])

            def mk_psum(i, nm):
                psA = psum_pool.tile([P, TA], f32, tag=f"psA{i}", name=f"psA{nm}")
                psB = (
                    psum_pool.tile([P, TB], f32, tag=f"psB{i}", name=f"psB{nm}")
                    if TB
                    else None
                )
                return psA, psB

            # ---- phase 0: o-tiles 0-3, k-outer (DMA-paced) ------------------
            ps = [mk_psum(i, f"p0_{i}") for i in range(4)]

            for _ in range(N_WARM):
                nc.tensor.matmul(
                    ps[0][0][:, :P],
                    lhsT=warm_sb[:],
                    rhs=warm_sb[:],
                    start=True,
                    stop=True,
                )

            for k in range(KO):
                for i in range(4):
                    lhsT = wsl(k, i)
                    nc.tensor.matmul(
                        ps[i][0][:],
                        lhsT=lhsT,
                        rhs=xsl(k, 0, TA),
                        start=(k == 0),
                        stop=(k == KO - 1),
                    )
                    if TB:
                        nc.tensor.matmul(
                            ps[i][1][:],
                            lhsT=lhsT,
                            rhs=xsl(k, TA, T_pad),
                            start=(k == 0),
                            stop=(k == KO - 1),
                        )
            for i in range(4):
                emit_out(i, ps[i][0], ps[i][1])

            # ---- phase 1: o-tiles 4-7, ot-outer (SBUF-resident) -------------
            for i in range(4):
                ot = 4 + i
                psA, psB = mk_psum(i, f"p1_{i}")
                for k in range(KO):
                    lhsT = wsl(k, ot)
                    nc.tensor.matmul(
                        psA[:],
                        lhsT=lhsT,
                        rhs=xsl(k, 0, TA),
                        start=(k == 0),
                        stop=(k == KO - 1),
                    )
                    if TB:
                        nc.tensor.matmul(
                            psB[:],
                            lhsT=lhsT,
                            rhs=xsl(k, TA, T_pad),
                            start=(k == 0),
                            stop=(k == KO - 1),
                        )
                emit_out(ot, psA, psB)
    nc.finalize()
    return nc


def kernel(x, category_id, weight, bias):
    global LAST_RESULTS
    x = np.asarray(x)
    category_id = np.asarray(category_id)
    weight = np.ascontiguousarray(np.asarray(weight), dtype=np.float32)
    bias = np.ascontiguousarray(np.asarray(bias), dtype=np.float32)

    orig_shape = x.shape
    D = orig_shape[-1]
    C, _, O = weight.shape
    KO, OT = D // P, O // P
    assert C == N_CORES and KO == 8 and OT == 8

    T = int(np.prod(orig_shape[:-1]))
    x_flat = np.ascontiguousarray(x.reshape(T, D), dtype=np.float32)
    cid = category_id.reshape(T).astype(np.int64)

    idx_per_c = [np.flatnonzero(cid == c) for c in range(C)]
    counts = [len(ix) for ix in idx_per_c]
    T_pad = max(32, -(-max(counts) // 16) * 16)

    key = (T_pad, D, O)
    if key not in _nc_cache:
        _nc_cache[key] = _build_nc(T_pad, D, O)
    nc = _nc_cache[key]

    # pre-arranged per-partition-contiguous layouts (see _build_nc)
    in_maps = []
    for c in range(C):
        xc = np.zeros((T_pad, D), dtype=np.float32)
        xc[: counts[c]] = x_flat[idx_per_c[c]]
        # [t, (k p)] -> [p, k, t]
        xh = np.ascontiguousarray(
            xc.T.reshape(KO, P, T_pad).transpose(1, 0, 2), dtype=np.float32
        ).astype(BF16).reshape(P, KO * T_pad)
        # [(k p), (g i o)] -> [p, g, k, i, o]   (g = ot//4, i = ot%4)
        wh = np.ascontiguousarray(
            weight[c].reshape(KO, P, 2, 4, P).transpose(1, 2, 0, 3, 4),
            dtype=np.float32,
        ).astype(BF16).reshape(P, KO * O)
        # [(ot o)] -> [o, ot]
        bh = np.ascontiguousarray(bias[c].reshape(OT, P).T)
        in_maps.append({"x": xh, "w": wh, "bias": bh})

    res = run_bass_kernel_spmd(nc, in_maps, list(range(N_CORES)))
    LAST_RESULTS = res

    out_flat = np.empty((T, O), dtype=np.float32)
    for c in range(C):
        oc = np.asarray(res.results[c]["out"])  # [O, T_pad] bf16
        out_flat[idx_per_c[c]] = oc[:, : counts[c]].T.astype(np.float32)
    return out_flat.reshape(*orig_shape[:-1], O)


# revision 8
# speedup vs baseline: 1.2199x; 1.0191x over previous
"""CategorySpecificLinear Trainium2 kernel (v3: bf16 weight-stationary).

out[t] = x[t] @ weight[category_id[t]] + bias[category_id[t]]

Strategy: expert-parallel over the 8 categories (C == n_cores == 8).
Host routes tokens by category; core c computes its category's tokens.

Device-side formulation (per core, transposed output):
    outT[o, t] = sum_k wT[k, o] * xT[k, t] + bias[o]
with the weight tile [128k x 128o] STATIONARY in the PE array and x
streamed as the moving operand, so PE stream cycles = OT*KO*T_pad
(~34k cycles ~ 14.1 us warm @2.4GHz) with zero m-tile quantization
waste. All operands bf16 (psum accumulates fp32) -> half the HBM
traffic of fp32 and 1 col/cycle warm on the PE.

Schedule (8 psum banks = 4 o-tiles x 2 T-halves in flight):
  phase 0 (o-tiles 0-3): k-OUTER -- each k-step needs only x[k] and
    w[g0,k], so the PE starts ~1.5 us after the first small chunks
    land instead of after the full 3 MB input load.
  phase 1 (o-tiles 4-7): everything is SBUF-resident by now, so run
    ot-OUTER: each o-tile's output drains (DVE/ACT bias-add + bf16
    store) while the next o-tile computes -> only the last o-tile's
    add+store is kernel tail.
Warm-up: ~24 dummy matmuls bridge the initial DMA wait so the HAM
clock gate reaches 8/8 before the real stream.

DMA: three queues (sync + scalar HWDGE, gpsimd SWDGE) each see
~190 GB/s when all active and lose ~1 us between chained DMAs, so
inputs are split in 2 chunks per queue (small first chunk for early
PE start): sync = x, scalar = w[o-tiles 0-3], gpsimd = bias + w[o-
tiles 4-7]. Host pre-arranges x as [p][k][t] and w as [p][g][k][o]
(per-partition contiguous) so every load is a plain 2D slice.
Per-core HBM: x 1.06 + w 2 + out 1.06 MB = 4.2 MB.

bf16 numerics: rel err ~3e-3 on dot-1024 (gate is 2e-2).
"""

import contextlib
import ctypes
import os
import sys
import types

import numpy as np

sys.path.insert(0, "/opt/trn_rl_repo")


def _ensure_ntff_hook():
    """Provide antenv.axon_hooks if the image lacks it.

    concourse.bass_utils imports antenv.axon_hooks.get_axon_ntff_profile_hook
    when trace=True under axon; some agent images don't ship that module, in
    which case the boot's NTFF hook registration silently degrades and the
    import in bass_utils crashes. Recreate the slim ctypes hook here
    (mirrors trn_agent_boot.trn_boot._ntff_profile_via_ctypes).
    """
    try:
        import antenv.axon_hooks  # noqa: F401

        return
    except ImportError:
        pass

    so_path = "/opt/axon/libaxon_pjrt.so"
    hook = None
    if os.path.exists(so_path):
        lib = ctypes.CDLL(so_path)
        if hasattr(lib, "axon_start_nrt_profile"):
            lib.axon_start_nrt_profile.argtypes = [
                ctypes.POINTER(ctypes.c_int64),
                ctypes.c_size_t,
            ]
            lib.axon_start_nrt_profile.restype = ctypes.c_int64
            lib.axon_stop_nrt_profile.argtypes = [ctypes.c_char_p]
            lib.axon_stop_nrt_profile.restype = ctypes.c_int64

            @contextlib.contextmanager
            def hook(output_dir, device_ids):
                import jax

                jax.devices()
                if device_ids:
                    ids = (ctypes.c_int64 * len(device_ids))(*device_ids)
                    rc = lib.axon_start_nrt_profile(ids, len(device_ids))
                else:
                    rc = lib.axon_start_nrt_profile(None, 0)
                if rc != 0:
                    raise RuntimeError(f"axon_start_nrt_profile rc={rc}")
                try:
                    yield
                finally:
                    n = lib.axon_stop_nrt_profile(str(output_dir).encode())
                    if n <= 0:
                        print(
                            f"ntff profile: rc={n} writing {output_dir}",
                            file=sys.stderr,
                        )

    mod = types.ModuleType("antenv.axon_hooks")
    _state = {"hook": hook}
    mod.set_axon_ntff_profile_hook = lambda h: _state.__setitem__("hook", h)
    mod.get_axon_ntff_profile_hook = lambda: _state["hook"]
    sys.modules["antenv.axon_hooks"] = mod
    try:
        import antenv

        antenv.axon_hooks = mod
    except ImportError:
        pass


_ensure_ntff_hook()

import ml_dtypes

import concourse.bass as bass
import concourse.bacc as bacc_mod
import concourse.mybir as mybir
import concourse.tile as tile
from concourse.bass_utils import run_bass_kernel_spmd

N_CORES = 8
P = 128
BF16 = np.dtype(ml_dtypes.bfloat16)

_nc_cache = {}
LAST_RESULTS = None  # BassKernelResults of the most recent run (for test.py)

N_WARM = 20  # dummy matmuls bridging the initial DMA wait (HAM warm-up)


def _build_nc(T_pad: int, D: int, O: int):
    KO = D // P
    OT = O // P
    assert KO == 8 and OT == 8
    bf = mybir.dt.bfloat16
    f32 = mybir.dt.float32

    # moving-operand split: one psum bank holds <=512 fp32 per partition,
    # so stream T in two halves (both multiples of 16)
    if T_pad <= 512:
        TA, TB = T_pad, 0
    else:
        TA = -(-(T_pad // 2) // 16) * 16
        TB = T_pad - TA
        assert TB <= 512

    GW = 4 * KO * P  # elems per ot-group per partition in w (4096)

    nc = bacc_mod.Bacc()
    x = nc.dram_tensor("x", [P, KO * T_pad], bf, kind="ExternalInput")
    w = nc.dram_tensor("w", [P, 2 * GW], bf, kind="ExternalInput")
    bias = nc.dram_tensor("bias", [P, OT], f32, kind="ExternalInput")
    out = nc.dram_tensor("out", [O, T_pad], bf, kind="ExternalOutput")

    # chunk boundaries (k-index splits) per queue: tiny first chunks so
    # the PE starts ~1.5us after first issue, then growing chunks to
    # amortize the ~1us inter-DMA ring gap. w for o-tiles 4-7 (phase 1)
    # is deferred: k0-3 on gpsimd, k4-7 queued on scalar BEHIND the
    # phase-0 weights so it never competes with phase-0's data.
    XSPL = [0, 1, 3, 5, KO]  # x on sync
    WSPL = [0, 1, 4, KO]  # w g0 on scalar
    GSPL = 4  # w g1: k0-3 on gpsimd, k4-7 appended on scalar

    with tile.TileContext(nc) as tc:
        with (
            tc.tile_pool(name="resident", bufs=1) as rpool,
            tc.tile_pool(name="psum", bufs=1, space="PSUM") as psum_pool,
            tc.tile_pool(name="obuf", bufs=4) as opool,
        ):
            # ---- input DMAs -------------------------------------------------
            x_sb = rpool.tile([P, KO * T_pad], bf, tag="x")
            for a, b in zip(XSPL, XSPL[1:]):
                nc.sync.dma_start(
                    x_sb[:, a * T_pad : b * T_pad], x[:, a * T_pad : b * T_pad]
                )

            w_sb = rpool.tile([P, 2 * GW], bf, tag="w")
            for a, b in zip(WSPL, WSPL[1:]):
                nc.scalar.dma_start(
                    w_sb[:, a * 4 * P : b * 4 * P], w[:, a * 4 * P : b * 4 * P]
                )

            bias_sb = rpool.tile([P, OT], f32, tag="bias")
            nc.gpsimd.dma_start(bias_sb[:], bias[:, :])
            c2 = GW + GSPL * 4 * P
            nc.gpsimd.dma_start(w_sb[:, GW:c2], w[:, GW:c2])
            nc.scalar.dma_start(w_sb[:, c2:], w[:, c2:])

            def wsl(k, ot):
                g, i = ot // 4, ot % 4
                base = g * GW + k * 4 * P + i * P
                return w_sb[:, base : base + P]

            def xsl(k, lo, hi):
                return x_sb[:, k * T_pad + lo : k * T_pad + hi]

            # ---- PE warm-up -------------------------------------------------
            # HAM un-throttles after ~3.4us of sustained PE activity; dummy
            # matmuls during the initial DMA wait start that clock early.
            # Results land in psA0 which the first real MM (start=True)
            # overwrites.
            warm_sb = rpool.tile([P, P], bf, tag="warm")
            nc.vector.memset(warm_sb[:], 0.0)

            def emit_out(ot, psA, psB):
                o_sb = opool.tile([P, T_pad], bf, tag="ot", name=f"o{ot}")
                bcol = bias_sb[:, ot : ot + 1]
                nc.vector.tensor_scalar_add(o_sb[:, :TA], psA[:], bcol)
                if TB:
                    nc.scalar.activation(
                        o_sb[:, TA:],
                        psB[:],
                        mybir.ActivationFunctionType.Identity,
                        bias=bcol,
                        scale=1.0,
                    )
                eng = nc.sync if ot % 2 == 0 else nc.scalar
                eng.dma_start(out[ot * P : (ot + 1) * P, :], o_sb[:])

            def mk_psum(i, nm):
                psA = psum_pool.tile([P, TA], f32, tag=f"psA{i}", name=f"psA{nm}")
                psB = (
                    psum_pool.tile([P, TB], f32, tag=f"psB{i}", name=f"psB{nm}")
                    if TB
                    else None
                )
                return psA, psB

            # ---- phase 0: o-tiles 0-3, k-outer (DMA-paced) ------------------
            ps = [mk_psum(i, f"p0_{i}") for i in range(4)]

            for _ in range(N_WARM):
                nc.tensor.matmul(
                    ps[0][0][:, :P],
                    lhsT=warm_sb[:],
                    rhs=warm_sb[:],
                    start=True,
                    stop=True,
                )

            for k in range(KO):
                for i in range(4):
                    lhsT = wsl(k, i)
                    nc.tensor.matmul(
                        ps[i][0][:],
                        lhsT=lhsT,
                        rhs=xsl(k, 0, TA),
                        start=(k == 0),
                        stop=(k == KO - 1),
                    )
                    if TB:
                        nc.tensor.matmul(
                            ps[i][1][:],
                            lhsT=lhsT,
                            rhs=xsl(k, TA, T_pad),
                            start=(k == 0),
                            stop=(k == KO - 1),
                        )
            for i in range(4):
                emit_out(i, ps[i][0], ps[i][1])

            # ---- phase 1: o-tiles 4-7, ot-outer (SBUF-resident) -------------
            # Per-half psum groups: the A-half's bias-add runs on DVE while
            # the B-half's matmuls stream, so only the final half's add +
            # store is kernel tail. The last o-tile's store is split across
            # both HWDGE queues to halve its wire+issue time.
            for i in range(4):
                ot = 4 + i
                psA, psB = mk_psum(i, f"p1_{i}")
                o_sb = opool.tile([P, T_pad], bf, tag="ot", name=f"o{ot}")
                bcol = bias_sb[:, ot : ot + 1]
                for k in range(KO):
                    nc.tensor.matmul(
                        psA[:],
                        lhsT=wsl(k, ot),
                        rhs=xsl(k, 0, TA),
                        start=(k == 0),
                        stop=(k == KO - 1),
                    )
                nc.vector.tensor_scalar_add(o_sb[:, :TA], psA[:], bcol)
                if TB:
                    for k in range(KO):
                        nc.tensor.matmul(
                            psB[:],
                            lhsT=wsl(k, ot),
                            rhs=xsl(k, TA, T_pad),
                            start=(k == 0),
                            stop=(k == KO - 1),
                        )
                    nc.scalar.activation(
                        o_sb[:, TA:],
                        psB[:],
                        mybir.ActivationFunctionType.Identity,
                        bias=bcol,
                        scale=1.0,
                    )
                orow = out[ot * P : (ot + 1) * P, :]
                if ot == OT - 1 and TB:
                    nc.sync.dma_start(orow[:, :TA], o_sb[:, :TA])
                    nc.scalar.dma_start(orow[:, TA:], o_sb[:, TA:])
                else:
                    eng = nc.sync if ot % 2 == 0 else nc.scalar
                    eng.dma_start(orow, o_sb[:])
    nc.finalize()
    return nc


def kernel(x, category_id, weight, bias):
    global LAST_RESULTS
    x = np.asarray(x)
    category_id = np.asarray(category_id)
    weight = np.ascontiguousarray(np.asarray(weight), dtype=np.float32)
    bias = np.ascontiguousarray(np.asarray(bias), dtype=np.float32)

    orig_shape = x.shape
    D = orig_shape[-1]
    C, _, O = weight.shape
    KO, OT = D // P, O // P
    assert C == N_CORES and KO == 8 and OT == 8

    T = int(np.prod(orig_shape[:-1]))
    x_flat = np.ascontiguousarray(x.reshape(T, D), dtype=np.float32)
    cid = category_id.reshape(T).astype(np.int64)

    idx_per_c = [np.flatnonzero(cid == c) for c in range(C)]
    counts = [len(ix) for ix in idx_per_c]
    T_pad = max(32, -(-max(counts) // 16) * 16)

    key = (T_pad, D, O)
    if key not in _nc_cache:
        _nc_cache[key] = _build_nc(T_pad, D, O)
    nc = _nc_cache[key]

    # pre-arranged per-partition-contiguous layouts (see _build_nc)
    in_maps = []
    for c in range(C):
        xc = np.zeros((T_pad, D), dtype=np.float32)
        xc[: counts[c]] = x_flat[idx_per_c[c]]
        # [t, (k p)] -> [p, k, t]
        xh = np.ascontiguousarray(
            xc.T.reshape(KO, P, T_pad).transpose(1, 0, 2), dtype=np.float32
        ).astype(BF16).reshape(P, KO * T_pad)
        # [(k p), (g i o)] -> [p, g, k, i, o]   (g = ot//4, i = ot%4)
        wh = np.ascontiguousarray(
            weight[c].reshape(KO, P, 2, 4, P).transpose(1, 2, 0, 3, 4),
            dtype=np.float32,
        ).astype(BF16).reshape(P, KO * O)
        # [(ot o)] -> [o, ot]
        bh = np.ascontiguousarray(bias[c].reshape(OT, P).T)
        in_maps.append({"x": xh, "w": wh, "bias": bh})

    res = run_bass_kernel_spmd(nc, in_maps, list(range(N_CORES)))
    LAST_RESULTS = res

    out_flat = np.empty((T, O), dtype=np.float32)
    for c in range(C):
        oc = np.asarray(res.results[c]["out"])  # [O, T_pad] bf16
        out_flat[idx_per_c[c]] = oc[:, : counts[c]].T.astype(np.float32)
    return out_flat.reshape(*orig_shape[:-1], O)


# revision 14
# speedup vs baseline: 1.3185x; 1.0808x over previous
"""CategorySpecificLinear Trainium2 kernel (v3: bf16 weight-stationary).

out[t] = x[t] @ weight[category_id[t]] + bias[category_id[t]]

Strategy: expert-parallel over the 8 categories (C == n_cores == 8).
Host routes tokens by category; core c computes its category's tokens.

Device-side formulation (per core, transposed output):
    outT[o, t] = sum_k wT[k, o] * xT[k, t] + bias[o]
with the weight tile [128k x 128o] STATIONARY in the PE array and x
streamed as the moving operand, so PE stream cycles = OT*KO*T_pad
(~34k cycles ~ 14.1 us warm @2.4GHz) with zero m-tile quantization
waste. All operands bf16 (psum accumulates fp32) -> half the HBM
traffic of fp32 and 1 col/cycle warm on the PE.

Schedule (8 psum banks = 4 o-tiles x 2 T-halves in flight):
  phase 0 (o-tiles 0-3): k-OUTER -- each k-step needs only x[k] and
    w[g0,k], so the PE starts ~1.5 us after the first small chunks
    land instead of after the full 3 MB input load.
  phase 1 (o-tiles 4-7): everything is SBUF-resident by now, so run
    ot-OUTER: each o-tile's output drains (DVE/ACT bias-add + bf16
    store) while the next o-tile computes -> only the last o-tile's
    add+store is kernel tail.
Warm-up: ~24 dummy matmuls bridge the initial DMA wait so the HAM
clock gate reaches 8/8 before the real stream.

DMA: three queues (sync + scalar HWDGE, gpsimd SWDGE) each see
~190 GB/s when all active and lose ~1 us between chained DMAs, so
inputs are split in 2 chunks per queue (small first chunk for early
PE start): sync = x, scalar = w[o-tiles 0-3], gpsimd = bias + w[o-
tiles 4-7]. Host pre-arranges x as [p][k][t] and w as [p][g][k][o]
(per-partition contiguous) so every load is a plain 2D slice.
Per-core HBM: x 1.06 + w 2 + out 1.06 MB = 4.2 MB.

bf16 numerics: rel err ~3e-3 on dot-1024 (gate is 2e-2).
"""

import contextlib
import ctypes
import os
import sys
import types

import numpy as np

sys.path.insert(0, "/opt/trn_rl_repo")


def _ensure_ntff_hook():
    """Provide antenv.axon_hooks if the image lacks it.

    concourse.bass_utils imports antenv.axon_hooks.get_axon_ntff_profile_hook
    when trace=True under axon; some agent images don't ship that module, in
    which case the boot's NTFF hook registration silently degrades and the
    import in bass_utils crashes. Recreate the slim ctypes hook here
    (mirrors trn_agent_boot.trn_boot._ntff_profile_via_ctypes).
    """
    try:
        import antenv.axon_hooks  # noqa: F401

        return
    except ImportError:
        pass

    so_path = "/opt/axon/libaxon_pjrt.so"
    hook = None
    if os.path.exists(so_path):
        lib = ctypes.CDLL(so_path)
        if hasattr(lib, "axon_start_nrt_profile"):
            lib.axon_start_nrt_profile.argtypes = [
                ctypes.POINTER(ctypes.c_int64),
                ctypes.c_size_t,
            ]
            lib.axon_start_nrt_profile.restype = ctypes.c_int64
            lib.axon_stop_nrt_profile.argtypes = [ctypes.c_char_p]
            lib.axon_stop_nrt_profile.restype = ctypes.c_int64

            @contextlib.contextmanager
            def hook(output_dir, device_ids):
                import jax

                jax.devices()
                if device_ids:
                    ids = (ctypes.c_int64 * len(device_ids))(*device_ids)
                    rc = lib.axon_start_nrt_profile(ids, len(device_ids))
                else:
                    rc = lib.axon_start_nrt_profile(None, 0)
                if rc != 0:
                    raise RuntimeError(f"axon_start_nrt_profile rc={rc}")
                try:
                    yield
                finally:
                    n = lib.axon_stop_nrt_profile(str(output_dir).encode())
                    if n <= 0:
                        print(
                            f"ntff profile: rc={n} writing {output_dir}",
                            file=sys.stderr,
                        )

    mod = types.ModuleType("antenv.axon_hooks")
    _state = {"hook": hook}
    mod.set_axon_ntff_profile_hook = lambda h: _state.__setitem__("hook", h)
    mod.get_axon_ntff_profile_hook = lambda: _state["hook"]
    sys.modules["antenv.axon_hooks"] = mod
    try:
        import antenv

        antenv.axon_hooks = mod
    except ImportError:
        pass


_ensure_ntff_hook()

import ml_dtypes

import concourse.bass as bass
import concourse.bacc as bacc_mod
import concourse.mybir as mybir
import concourse.tile as tile
from concourse.bass_utils import run_bass_kernel_spmd

N_CORES = 8
P = 128
BF16 = np.dtype(ml_dtypes.bfloat16)

_nc_cache = {}
LAST_RESULTS = None  # BassKernelResults of the most recent run (for test.py)

N_WARM = 25  # dummy matmuls bridging the initial DMA wait (HAM warm-up)


def _build_nc(T_pad: int, D: int, O: int):
    KO = D // P
    OT = O // P
    assert KO == 8 and OT == 8
    bf = mybir.dt.bfloat16
    f32 = mybir.dt.float32

    # moving-operand split: one psum bank holds <=512 fp32 per partition,
    # so stream T in two halves (both multiples of 16)
    if T_pad <= 512:
        TA, TB = T_pad, 0
    else:
        TA = -(-(T_pad // 2) // 16) * 16
        TB = T_pad - TA
        assert TB <= 512

    GW = 4 * KO * P  # elems per ot-group per partition in w (4096)

    nc = bacc_mod.Bacc()
    x = nc.dram_tensor("x", [P, KO * T_pad], bf, kind="ExternalInput")
    w = nc.dram_tensor("w", [P, 2 * GW], bf, kind="ExternalInput")
    bias = nc.dram_tensor("bias", [P, OT], f32, kind="ExternalInput")
    out = nc.dram_tensor("out", [O, T_pad], bf, kind="ExternalOutput")

    # The DMA rings round-robin fairly at packet granularity, so ring
    # FIFO order is the only priority mechanism: phase-0's data (x on
    # sync, w[g0] on scalar, in k-need order with tiny first chunks) is
    # followed by the phase-1 weights on the SAME rings' tails, so wg1
    # never steals bandwidth from the k-paced phase-0 stream. gpsimd
    # carries only bias + the non-critical phase-0 output stores.
    XSPL = [0, 1, 3, 5, KO]  # x chunks on sync
    WSPL = [0, 1, 3, 5, KO]  # w g0 chunks on scalar
    GSPL = 4  # w g1: k0-3 appended on sync, k4-7 on scalar

    with tile.TileContext(nc) as tc:
        with (
            tc.tile_pool(name="resident", bufs=1) as rpool,
            tc.tile_pool(name="psum", bufs=1, space="PSUM") as psum_pool,
            tc.tile_pool(name="obuf", bufs=8) as opool,
        ):
            # ---- input DMAs -------------------------------------------------
            x_sb = rpool.tile([P, KO * T_pad], bf, tag="x")
            for a, b in zip(XSPL, XSPL[1:]):
                nc.sync.dma_start(
                    x_sb[:, a * T_pad : b * T_pad], x[:, a * T_pad : b * T_pad]
                )

            w_sb = rpool.tile([P, 2 * GW], bf, tag="w")
            for a, b in zip(WSPL, WSPL[1:]):
                nc.scalar.dma_start(
                    w_sb[:, a * 4 * P : b * 4 * P], w[:, a * 4 * P : b * 4 * P]
                )

            bias_sb = rpool.tile([P, OT], f32, tag="bias")
            nc.gpsimd.dma_start(bias_sb[:], bias[:, :])
            c2 = GW + GSPL * 4 * P
            nc.sync.dma_start(w_sb[:, GW:c2], w[:, GW:c2])
            nc.scalar.dma_start(w_sb[:, c2:], w[:, c2:])

            def wsl(k, ot):
                g, i = ot // 4, ot % 4
                base = g * GW + k * 4 * P + i * P
                return w_sb[:, base : base + P]

            def xsl(k, lo, hi):
                return x_sb[:, k * T_pad + lo : k * T_pad + hi]

            # ---- PE warm-up -------------------------------------------------
            # HAM un-throttles after ~3.4us of sustained PE activity; dummy
            # matmuls during the initial DMA wait start that clock early.
            # Results land in psA0 which the first real MM (start=True)
            # overwrites.
            warm_sb = rpool.tile([P, P], bf, tag="warm")
            nc.vector.memset(warm_sb[:], 0.0)

            def emit_out(ot, psA, psB):
                o_sb = opool.tile([P, T_pad], bf, tag="ot", name=f"o{ot}")
                bcol = bias_sb[:, ot : ot + 1]
                nc.vector.tensor_scalar_add(o_sb[:, :TA], psA[:], bcol)
                if TB:
                    nc.scalar.activation(
                        o_sb[:, TA:],
                        psB[:],
                        mybir.ActivationFunctionType.Identity,
                        bias=bcol,
                        scale=1.0,
                    )
                nc.gpsimd.dma_start(out[ot * P : (ot + 1) * P, :], o_sb[:])

            def mk_psum(i, nm):
                psA = psum_pool.tile([P, TA], f32, tag=f"psA{i}", name=f"psA{nm}")
                psB = (
                    psum_pool.tile([P, TB], f32, tag=f"psB{i}", name=f"psB{nm}")
                    if TB
                    else None
                )
                return psA, psB

            # ---- phase 0: o-tiles 0-3, k-outer (DMA-paced) ------------------
            ps = [mk_psum(i, f"p0_{i}") for i in range(4)]

            for _ in range(N_WARM):
                nc.tensor.matmul(
                    ps[0][0][:, :P],
                    lhsT=warm_sb[:],
                    rhs=warm_sb[:],
                    start=True,
                    stop=True,
                )

            for k in range(KO):
                for i in range(4):
                    lhsT = wsl(k, i)
                    nc.tensor.matmul(
                        ps[i][0][:],
                        lhsT=lhsT,
                        rhs=xsl(k, 0, TA),
                        start=(k == 0),
                        stop=(k == KO - 1),
                    )
                    if TB:
                        nc.tensor.matmul(
                            ps[i][1][:],
                            lhsT=lhsT,
                            rhs=xsl(k, TA, T_pad),
                            start=(k == 0),
                            stop=(k == KO - 1),
                        )
            for i in range(4):
                emit_out(i, ps[i][0], ps[i][1])

            # ---- phase 1: o-tiles 4-7, ot-outer (SBUF-resident) -------------
            # Per-half psum groups: the A-half's bias-add runs on DVE while
            # the B-half's matmuls stream, so only the final half's add +
            # store is kernel tail. The last o-tile's store is split across
            # both HWDGE queues to halve its wire+issue time.
            for i in range(4):
                ot = 4 + i
                psA, psB = mk_psum(i, f"p1_{i}")
                o_sb = opool.tile([P, T_pad], bf, tag="ot", name=f"o{ot}")
                bcol = bias_sb[:, ot : ot + 1]
                for k in range(KO):
                    nc.tensor.matmul(
                        psA[:],
                        lhsT=wsl(k, ot),
                        rhs=xsl(k, 0, TA),
                        start=(k == 0),
                        stop=(k == KO - 1),
                    )
                nc.vector.tensor_scalar_add(o_sb[:, :TA], psA[:], bcol)
                if TB:
                    for k in range(KO):
                        nc.tensor.matmul(
                            psB[:],
                            lhsT=wsl(k, ot),
                            rhs=xsl(k, TA, T_pad),
                            start=(k == 0),
                            stop=(k == KO - 1),
                        )
                    nc.scalar.activation(
                        o_sb[:, TA:],
                        psB[:],
                        mybir.ActivationFunctionType.Identity,
                        bias=bcol,
                        scale=1.0,
                    )
                orow = out[ot * P : (ot + 1) * P, :]
                if ot == OT - 1 and TB:
                    # critical tail: halves in parallel on the (now idle)
                    # HWDGE rings
                    nc.sync.dma_start(orow[:, :TA], o_sb[:, :TA])
                    nc.scalar.dma_start(orow[:, TA:], o_sb[:, TA:])
                elif ot == OT - 2:
                    nc.sync.dma_start(orow, o_sb[:])
                else:
                    nc.gpsimd.dma_start(orow, o_sb[:])
    nc.finalize()
    return nc


def kernel(x, category_id, weight, bias):
    global LAST_RESULTS
    x = np.asarray(x)
    category_id = np.asarray(category_id)
    weight = np.ascontiguousarray(np.asarray(weight), dtype=np.float32)
    bias = np.ascontiguousarray(np.asarray(bias), dtype=np.float32)

    orig_shape = x.shape
    D = orig_shape[-1]
    C, _, O = weight.shape
    KO, OT = D // P, O // P
    assert C == N_CORES and KO == 8 and OT == 8

    T = int(np.prod(orig_shape[:-1]))
    x_flat = np.ascontiguousarray(x.reshape(T, D), dtype=np.float32)
    cid = category_id.reshape(T).astype(np.int64)

    idx_per_c = [np.flatnonzero(cid == c) for c in range(C)]
    counts = [len(ix) for ix in idx_per_c]
    T_pad = max(32, -(-max(counts) // 16) * 16)

    key = (T_pad, D, O)
    if key not in _nc_cache:
        _nc_cache[key] = _build_nc(T_pad, D, O)
    nc = _nc_cache[key]

    # pre-arranged per-partition-contiguous layouts (see _build_nc)
    in_maps = []
    for c in range(C):
        xc = np.zeros((T_pad, D), dtype=np.float32)
        xc[: counts[c]] = x_flat[idx_per_c[c]]
        # [t, (k p)] -> [p, k, t]
        xh = np.ascontiguousarray(
            xc.T.reshape(KO, P, T_pad).transpose(1, 0, 2), dtype=np.float32
        ).astype(BF16).reshape(P, KO * T_pad)
        # [(k p), (g i o)] -> [p, g, k, i, o]   (g = ot//4, i = ot%4)
        wh = np.ascontiguousarray(
            weight[c].reshape(KO, P, 2, 4, P).transpose(1, 2, 0, 3, 4),
            dtype=np.float32,
        ).astype(BF16).reshape(P, KO * O)
        # [(ot o)] -> [o, ot]
        bh = np.ascontiguousarray(bias[c].reshape(OT, P).T)
        in_maps.append({"x": xh, "w": wh, "bias": bh})

    res = run_bass_kernel_spmd(nc, in_maps, list(range(N_CORES)))
    LAST_RESULTS = res

    out_flat = np.empty((T, O), dtype=np.float32)
    for c in range(C):
        oc = np.asarray(res.results[c]["out"])  # [O, T_pad] bf16
        out_flat[idx_per_c[c]] = oc[:, : counts[c]].T.astype(np.float32)
    return out_flat.reshape(*orig_shape[:-1], O)


# revision 17
# speedup vs baseline: 1.4097x; 1.0692x over previous
"""CategorySpecificLinear Trainium2 kernel (v3: bf16 weight-stationary).

out[t] = x[t] @ weight[category_id[t]] + bias[category_id[t]]

Strategy: expert-parallel over the 8 categories (C == n_cores == 8).
Host routes tokens by category; core c computes its category's tokens.

Device-side formulation (per core, transposed output):
    outT[o, t] = sum_k wT[k, o] * xT[k, t] + bias[o]
with the weight tile [128k x 128o] STATIONARY in the PE array and x
streamed as the moving operand, so PE stream cycles = OT*KO*T_pad
(~34k cycles ~ 14.1 us warm @2.4GHz) with zero m-tile quantization
waste. All operands bf16 (psum accumulates fp32) -> half the HBM
traffic of fp32 and 1 col/cycle warm on the PE.

Schedule (8 psum banks = 4 o-tiles x 2 T-halves in flight):
  phase 0 (o-tiles 0-3): k-OUTER -- each k-step needs only x[k] and
    w[g0,k], so the PE starts ~1.5 us after the first small chunks
    land instead of after the full 3 MB input load.
  phase 1 (o-tiles 4-7): everything is SBUF-resident by now, so run
    ot-OUTER: each o-tile's output drains (DVE/ACT bias-add + bf16
    store) while the next o-tile computes -> only the last o-tile's
    add+store is kernel tail.
Warm-up: ~24 dummy matmuls bridge the initial DMA wait so the HAM
clock gate reaches 8/8 before the real stream.

DMA: three queues (sync + scalar HWDGE, gpsimd SWDGE) each see
~190 GB/s when all active and lose ~1 us between chained DMAs, so
inputs are split in 2 chunks per queue (small first chunk for early
PE start): sync = x, scalar = w[o-tiles 0-3], gpsimd = bias + w[o-
tiles 4-7]. Host pre-arranges x as [p][k][t] and w as [p][g][k][o]
(per-partition contiguous) so every load is a plain 2D slice.
Per-core HBM: x 1.06 + w 2 + out 1.06 MB = 4.2 MB.

bf16 numerics: rel err ~3e-3 on dot-1024 (gate is 2e-2).
"""

import contextlib
import ctypes
import os
import sys
import types

import numpy as np

sys.path.insert(0, "/opt/trn_rl_repo")


def _ensure_ntff_hook():
    """Provide antenv.axon_hooks if the image lacks it.

    concourse.bass_utils imports antenv.axon_hooks.get_axon_ntff_profile_hook
    when trace=True under axon; some agent images don't ship that module, in
    which case the boot's NTFF hook registration silently degrades and the
    import in bass_utils crashes. Recreate the slim ctypes hook here
    (mirrors trn_agent_boot.trn_boot._ntff_profile_via_ctypes).
    """
    try:
        import antenv.axon_hooks  # noqa: F401

        return
    except ImportError:
        pass

    so_path = "/opt/axon/libaxon_pjrt.so"
    hook = None
    if os.path.exists(so_path):
        lib = ctypes.CDLL(so_path)
        if hasattr(lib, "axon_start_nrt_profile"):
            lib.axon_start_nrt_profile.argtypes = [
                ctypes.POINTER(ctypes.c_int64),
                ctypes.c_size_t,
            ]
            lib.axon_start_nrt_profile.restype = ctypes.c_int64
            lib.axon_stop_nrt_profile.argtypes = [ctypes.c_char_p]
            lib.axon_stop_nrt_profile.restype = ctypes.c_int64

            @contextlib.contextmanager
            def hook(output_dir, device_ids):
                import jax

                jax.devices()
                if device_ids:
                    ids = (ctypes.c_int64 * len(device_ids))(*device_ids)
                    rc = lib.axon_start_nrt_profile(ids, len(device_ids))
                else:
                    rc = lib.axon_start_nrt_profile(None, 0)
                if rc != 0:
                    raise RuntimeError(f"axon_start_nrt_profile rc={rc}")
                try:
                    yield
                finally:
                    n = lib.axon_stop_nrt_profile(str(output_dir).encode())
                    if n <= 0:
                        print(
                            f"ntff profile: rc={n} writing {output_dir}",
                            file=sys.stderr,
                        )

    mod = types.ModuleType("antenv.axon_hooks")
    _state = {"hook": hook}
    mod.set_axon_ntff_profile_hook = lambda h: _state.__setitem__("hook", h)
    mod.get_axon_ntff_profile_hook = lambda: _state["hook"]
    sys.modules["antenv.axon_hooks"] = mod
    try:
        import antenv

        antenv.axon_hooks = mod
    except ImportError:
        pass


_ensure_ntff_hook()

import ml_dtypes

import concourse.bass as bass
import concourse.bacc as bacc_mod
import concourse.mybir as mybir
import concourse.tile as tile
from concourse.bass_utils import run_bass_kernel_spmd

N_CORES = 8
P = 128
BF16 = np.dtype(ml_dtypes.bfloat16)

_nc_cache = {}
LAST_RESULTS = None  # BassKernelResults of the most recent run (for test.py)

N_WARM = 25  # dummy matmuls bridging the initial DMA wait (HAM warm-up)


def _build_nc(T_pad: int, D: int, O: int):
    KO = D // P
    OT = O // P
    assert KO == 8 and OT == 8
    bf = mybir.dt.bfloat16
    f32 = mybir.dt.float32

    # moving-operand split: one psum bank holds <=512 fp32 per partition,
    # so stream T in two halves (both multiples of 16)
    if T_pad <= 512:
        TA, TB = T_pad, 0
    else:
        TA = -(-(T_pad // 2) // 16) * 16
        TB = T_pad - TA
        assert TB <= 512

    S = T_pad + 4 * P  # per-k elems per partition in xw (x slice + 4 o-tiles of w)
    GW = 4 * P  # per-k elems per partition in w1 (512)

    nc = bacc_mod.Bacc()
    xw = nc.dram_tensor("xw", [P, KO * S], bf, kind="ExternalInput")
    w1 = nc.dram_tensor("w1", [P, KO * GW], bf, kind="ExternalInput")
    bias = nc.dram_tensor("bias", [P, OT], f32, kind="ExternalInput")
    out = nc.dram_tensor("out", [O, T_pad], bf, kind="ExternalOutput")

    with tile.TileContext(nc) as tc:
        with (
            tc.tile_pool(name="resident", bufs=1) as rpool,
            tc.tile_pool(name="psum", bufs=1, space="PSUM") as psum_pool,
            tc.tile_pool(name="obuf", bufs=8) as opool,
        ):
            # ---- input DMAs -------------------------------------------------
            # The DMA rings round-robin fairly at packet granularity, so
            # ring FIFO order is the only priority mechanism. Phase 0's
            # data is ONE combined per-k stream (x slice + its 4 weight
            # tiles, exactly the PE's per-k consumption) alternated
            # across both HWDGE rings; k0/k1 are split x-vs-w across the
            # rings so the pipeline fills fast. The phase-1 weights ride
            # the same rings' tails in k order, so they never steal
            # bandwidth from the k-paced phase-0 stream. gpsimd carries
            # bias + the non-critical output stores.
            xw_sb = rpool.tile([P, KO * S], bf, tag="xw")

            def ldxw(eng, lo, hi):
                eng.dma_start(xw_sb[:, lo:hi], xw[:, lo:hi])

            ldxw(nc.sync, 0, T_pad)  # x k0
            ldxw(nc.scalar, T_pad, S)  # w k0
            ldxw(nc.sync, S, S + T_pad)  # x k1
            ldxw(nc.scalar, S + T_pad, 2 * S)  # w k1
            for k in range(2, KO):
                ldxw(nc.sync if k % 2 == 0 else nc.scalar, k * S, (k + 1) * S)

            w1_sb = rpool.tile([P, KO * GW], bf, tag="w1")
            for j, eng in enumerate([nc.sync, nc.sync, nc.scalar, nc.scalar]):
                eng.dma_start(
                    w1_sb[:, j * 2 * GW : (j + 1) * 2 * GW],
                    w1[:, j * 2 * GW : (j + 1) * 2 * GW],
                )

            bias_sb = rpool.tile([P, OT], f32, tag="bias")
            nc.gpsimd.dma_start(bias_sb[:], bias[:, :])

            def wsl(k, ot):
                if ot < 4:
                    base = k * S + T_pad + ot * P
                    return xw_sb[:, base : base + P]
                base = k * GW + (ot - 4) * P
                return w1_sb[:, base : base + P]

            def xsl(k, lo, hi):
                return xw_sb[:, k * S + lo : k * S + hi]

            # ---- PE warm-up -------------------------------------------------
            # HAM un-throttles after ~3.4us of sustained PE activity; dummy
            # matmuls during the initial DMA wait start that clock early.
            # Results land in psA0 which the first real MM (start=True)
            # overwrites.
            warm_sb = rpool.tile([P, P], bf, tag="warm")
            nc.vector.memset(warm_sb[:], 0.0)

            def emit_out(ot, psA, psB):
                o_sb = opool.tile([P, T_pad], bf, tag="ot", name=f"o{ot}")
                bcol = bias_sb[:, ot : ot + 1]
                nc.vector.tensor_scalar_add(o_sb[:, :TA], psA[:], bcol)
                if TB:
                    nc.scalar.activation(
                        o_sb[:, TA:],
                        psB[:],
                        mybir.ActivationFunctionType.Identity,
                        bias=bcol,
                        scale=1.0,
                    )
                nc.gpsimd.dma_start(out[ot * P : (ot + 1) * P, :], o_sb[:])

            def mk_psum(i, nm):
                psA = psum_pool.tile([P, TA], f32, tag=f"psA{i}", name=f"psA{nm}")
                psB = (
                    psum_pool.tile([P, TB], f32, tag=f"psB{i}", name=f"psB{nm}")
                    if TB
                    else None
                )
                return psA, psB

            # ---- phase 0: o-tiles 0-3, k-outer (DMA-paced) ------------------
            ps = [mk_psum(i, f"p0_{i}") for i in range(4)]

            for _ in range(N_WARM):
                nc.tensor.matmul(
                    ps[0][0][:, :P],
                    lhsT=warm_sb[:],
                    rhs=warm_sb[:],
                    start=True,
                    stop=True,
                )

            for k in range(KO):
                for i in range(4):
                    lhsT = wsl(k, i)
                    nc.tensor.matmul(
                        ps[i][0][:],
                        lhsT=lhsT,
                        rhs=xsl(k, 0, TA),
                        start=(k == 0),
                        stop=(k == KO - 1),
                    )
                    if TB:
                        nc.tensor.matmul(
                            ps[i][1][:],
                            lhsT=lhsT,
                            rhs=xsl(k, TA, T_pad),
                            start=(k == 0),
                            stop=(k == KO - 1),
                        )
            for i in range(4):
                emit_out(i, ps[i][0], ps[i][1])

            # ---- phase 1: o-tiles 4-7, ot-outer (SBUF-resident) -------------
            # Per-half psum groups: the A-half's bias-add runs on DVE while
            # the B-half's matmuls stream, so only the final half's add +
            # store is kernel tail. The last o-tile's store is split across
            # both HWDGE queues to halve its wire+issue time.
            for i in range(4):
                ot = 4 + i
                psA, psB = mk_psum(i, f"p1_{i}")
                o_sb = opool.tile([P, T_pad], bf, tag="ot", name=f"o{ot}")
                bcol = bias_sb[:, ot : ot + 1]
                last = ot == OT - 1
                orow = out[ot * P : (ot + 1) * P, :]
                for k in range(KO):
                    nc.tensor.matmul(
                        psA[:],
                        lhsT=wsl(k, ot),
                        rhs=xsl(k, 0, TA),
                        start=(k == 0),
                        stop=(k == KO - 1),
                    )
                nc.vector.tensor_scalar_add(o_sb[:, :TA], psA[:], bcol)
                if last and TB:
                    # critical tail: A half stores while the B half's
                    # matmuls stream; B's bias-add is split DVE/ACT and
                    # its store rides the other (idle) HWDGE ring
                    nc.sync.dma_start(orow[:, :TA], o_sb[:, :TA])
                if TB:
                    for k in range(KO):
                        nc.tensor.matmul(
                            psB[:],
                            lhsT=wsl(k, ot),
                            rhs=xsl(k, TA, T_pad),
                            start=(k == 0),
                            stop=(k == KO - 1),
                        )
                    if last:
                        h = TB // 2
                        nc.scalar.activation(
                            o_sb[:, TA : TA + h],
                            psB[:, :h],
                            mybir.ActivationFunctionType.Identity,
                            bias=bcol,
                            scale=1.0,
                        )
                        nc.vector.tensor_scalar_add(
                            o_sb[:, TA + h :], psB[:, h:], bcol
                        )
                        nc.scalar.dma_start(orow[:, TA:], o_sb[:, TA:])
                    else:
                        nc.scalar.activation(
                            o_sb[:, TA:],
                            psB[:],
                            mybir.ActivationFunctionType.Identity,
                            bias=bcol,
                            scale=1.0,
                        )
                if not last:
                    if ot == OT - 2:
                        nc.sync.dma_start(orow, o_sb[:])
                    else:
                        nc.gpsimd.dma_start(orow, o_sb[:])
                elif not TB:
                    nc.sync.dma_start(orow, o_sb[:])
    nc.finalize()
    return nc


def kernel(x, category_id, weight, bias):
    global LAST_RESULTS
    x = np.asarray(x)
    category_id = np.asarray(category_id)
    weight = np.ascontiguousarray(np.asarray(weight), dtype=np.float32)
    bias = np.ascontiguousarray(np.asarray(bias), dtype=np.float32)

    orig_shape = x.shape
    D = orig_shape[-1]
    C, _, O = weight.shape
    KO, OT = D // P, O // P
    assert C == N_CORES and KO == 8 and OT == 8

    T = int(np.prod(orig_shape[:-1]))
    x_flat = np.ascontiguousarray(x.reshape(T, D), dtype=np.float32)
    cid = category_id.reshape(T).astype(np.int64)

    idx_per_c = [np.flatnonzero(cid == c) for c in range(C)]
    counts = [len(ix) for ix in idx_per_c]
    T_pad = max(32, -(-max(counts) // 16) * 16)

    key = (T_pad, D, O)
    if key not in _nc_cache:
        _nc_cache[key] = _build_nc(T_pad, D, O)
    nc = _nc_cache[key]

    # pre-arranged per-partition-contiguous layouts (see _build_nc)
    in_maps = []
    for c in range(C):
        xc = np.zeros((T_pad, D), dtype=np.float32)
        xc[: counts[c]] = x_flat[idx_per_c[c]]
        # [t, (k p)] -> [p, k, t]
        xh = xc.T.reshape(KO, P, T_pad).transpose(1, 0, 2).astype(BF16)
        # [(k p), (g i o)] -> [g, p, k, i*o]   (g = ot//4, i = ot%4)
        wh = (
            weight[c]
            .reshape(KO, P, 2, 4 * P)
            .transpose(2, 1, 0, 3)
            .astype(BF16)
        )
        # combined per-k stream for phase 0: [p][k][x_t | w_g0]
        xwh = np.ascontiguousarray(
            np.concatenate([xh, wh[0]], axis=2)
        ).reshape(P, KO * (T_pad + 4 * P))
        w1h = np.ascontiguousarray(wh[1]).reshape(P, KO * 4 * P)
        # [(ot o)] -> [o, ot]
        bh = np.ascontiguousarray(bias[c].reshape(OT, P).T)
        in_maps.append({"xw": xwh, "w1": w1h, "bias": bh})

    res = run_bass_kernel_spmd(nc, in_maps, list(range(N_CORES)))
    LAST_RESULTS = res

    out_flat = np.empty((T, O), dtype=np.float32)
    for c in range(C):
        oc = np.asarray(res.results[c]["out"])  # [O, T_pad] bf16
        out_flat[idx_per_c[c]] = oc[:, : counts[c]].T.astype(np.float32)
    return out_flat.reshape(*orig_shape[:-1], O)


# revision 22
# speedup vs baseline: 1.4514x; 1.0296x over previous
"""CategorySpecificLinear Trainium2 kernel (v3: bf16 weight-stationary).

out[t] = x[t] @ weight[category_id[t]] + bias[category_id[t]]

Strategy: expert-parallel over the 8 categories (C == n_cores == 8).
Host routes tokens by category; core c computes its category's tokens.

Device-side formulation (per core, transposed output):
    outT[o, t] = sum_k wT[k, o] * xT[k, t] + bias[o]
with the weight tile [128k x 128o] STATIONARY in the PE array and x
streamed as the moving operand, so PE stream cycles = OT*KO*T_pad
(~34k cycles ~ 14.1 us warm @2.4GHz) with zero m-tile quantization
waste. All operands bf16 (psum accumulates fp32) -> half the HBM
traffic of fp32 and 1 col/cycle warm on the PE.

Schedule (8 psum banks = 4 o-tiles x 2 T-halves in flight):
  phase 0 (o-tiles 0-3): k-OUTER -- each k-step needs only x[k] and
    w[g0,k], so the PE starts ~1.5 us after the first small chunks
    land instead of after the full 3 MB input load.
  phase 1 (o-tiles 4-7): everything is SBUF-resident by now, so run
    ot-OUTER: each o-tile's output drains (DVE/ACT bias-add + bf16
    store) while the next o-tile computes -> only the last o-tile's
    add+store is kernel tail.
Warm-up: ~24 dummy matmuls bridge the initial DMA wait so the HAM
clock gate reaches 8/8 before the real stream.

DMA: three queues (sync + scalar HWDGE, gpsimd SWDGE) each see
~190 GB/s when all active and lose ~1 us between chained DMAs, so
inputs are split in 2 chunks per queue (small first chunk for early
PE start): sync = x, scalar = w[o-tiles 0-3], gpsimd = bias + w[o-
tiles 4-7]. Host pre-arranges x as [p][k][t] and w as [p][g][k][o]
(per-partition contiguous) so every load is a plain 2D slice.
Per-core HBM: x 1.06 + w 2 + out 1.06 MB = 4.2 MB.

bf16 numerics: rel err ~3e-3 on dot-1024 (gate is 2e-2).
"""

import contextlib
import ctypes
import os
import sys
import types

import numpy as np

sys.path.insert(0, "/opt/trn_rl_repo")


def _ensure_ntff_hook():
    """Provide antenv.axon_hooks if the image lacks it.

    concourse.bass_utils imports antenv.axon_hooks.get_axon_ntff_profile_hook
    when trace=True under axon; some agent images don't ship that module, in
    which case the boot's NTFF hook registration silently degrades and the
    import in bass_utils crashes. Recreate the slim ctypes hook here
    (mirrors trn_agent_boot.trn_boot._ntff_profile_via_ctypes).
    """
    try:
        import antenv.axon_hooks  # noqa: F401

        return
    except ImportError:
        pass

    so_path = "/opt/axon/libaxon_pjrt.so"
    hook = None
    if os.path.exists(so_path):
        lib = ctypes.CDLL(so_path)
        if hasattr(lib, "axon_start_nrt_profile"):
            lib.axon_start_nrt_profile.argtypes = [
                ctypes.POINTER(ctypes.c_int64),
                ctypes.c_size_t,
            ]
            lib.axon_start_nrt_profile.restype = ctypes.c_int64
            lib.axon_stop_nrt_profile.argtypes = [ctypes.c_char_p]
            lib.axon_stop_nrt_profile.restype = ctypes.c_int64

            @contextlib.contextmanager
            def hook(output_dir, device_ids):
                import jax

                jax.devices()
                if device_ids:
                    ids = (ctypes.c_int64 * len(device_ids))(*device_ids)
                    rc = lib.axon_start_nrt_profile(ids, len(device_ids))
                else:
                    rc = lib.axon_start_nrt_profile(None, 0)
                if rc != 0:
                    raise RuntimeError(f"axon_start_nrt_profile rc={rc}")
                try:
                    yield
                finally:
                    n = lib.axon_stop_nrt_profile(str(output_dir).encode())
                    if n <= 0:
                        print(
                            f"ntff profile: rc={n} writing {output_dir}",
                            file=sys.stderr,
                        )

    mod = types.ModuleType("antenv.axon_hooks")
    _state = {"hook": hook}
    mod.set_axon_ntff_profile_hook = lambda h: _state.__setitem__("hook", h)
    mod.get_axon_ntff_profile_hook = lambda: _state["hook"]
    sys.modules["antenv.axon_hooks"] = mod
    try:
        import antenv

        antenv.axon_hooks = mod
    except ImportError:
        pass


_ensure_ntff_hook()

import ml_dtypes

import concourse.bass as bass
import concourse.bacc as bacc_mod
import concourse.mybir as mybir
import concourse.tile as tile
from concourse.bass_utils import run_bass_kernel_spmd

N_CORES = 8
P = 128
BF16 = np.dtype(ml_dtypes.bfloat16)

_nc_cache = {}
LAST_RESULTS = None  # BassKernelResults of the most recent run (for test.py)

N_WARM = 20  # dummy matmuls bridging the initial DMA wait (HAM warm-up)


def _build_nc(T_pad: int, D: int, O: int):
    KO = D // P
    OT = O // P
    assert KO == 8 and OT == 8
    bf = mybir.dt.bfloat16
    f32 = mybir.dt.float32

    # moving-operand split: one psum bank holds <=512 fp32 per partition,
    # so stream T in two halves (both multiples of 16)
    if T_pad <= 512:
        TA, TB = T_pad, 0
    else:
        TA = -(-(T_pad // 2) // 16) * 16
        TB = T_pad - TA
        assert TB <= 512

    S = T_pad + 4 * P  # per-k elems per partition in xw (x slice + 4 o-tiles of w)
    GW = 4 * P  # per-k elems per partition in w1 (512)

    nc = bacc_mod.Bacc()
    xw = nc.dram_tensor("xw", [P, KO * S], bf, kind="ExternalInput")
    w1 = nc.dram_tensor("w1", [P, KO * GW], bf, kind="ExternalInput")
    bias = nc.dram_tensor("bias", [P, OT], f32, kind="ExternalInput")
    out = nc.dram_tensor("out", [O, T_pad], bf, kind="ExternalOutput")

    with tile.TileContext(nc) as tc:
        with (
            tc.tile_pool(name="resident", bufs=1) as rpool,
            tc.tile_pool(name="psum", bufs=1, space="PSUM") as psum_pool,
            tc.tile_pool(name="obuf", bufs=8) as opool,
        ):
            # ---- input DMAs -------------------------------------------------
            # The DMA rings round-robin fairly at packet granularity, so
            # ring FIFO order is the only priority mechanism. Phase 0's
            # data is ONE combined per-k stream (x slice + its 4 weight
            # tiles, exactly the PE's per-k consumption) alternated
            # across both HWDGE rings; k0/k1 are split x-vs-w across the
            # rings so the pipeline fills fast. The phase-1 weights ride
            # the same rings' tails in k order, so they never steal
            # bandwidth from the k-paced phase-0 stream. gpsimd carries
            # bias + the non-critical output stores.
            xw_sb = rpool.tile([P, KO * S], bf, tag="xw")

            def ldxw(eng, lo, hi):
                eng.dma_start(xw_sb[:, lo:hi], xw[:, lo:hi])

            ldxw(nc.sync, 0, T_pad)  # x k0
            ldxw(nc.scalar, T_pad, S)  # w k0
            ldxw(nc.gpsimd, S, 2 * S)  # k1 block rides the third (SWDGE) path
            for k in range(2, KO):
                ldxw(nc.sync if k % 2 == 0 else nc.scalar, k * S, (k + 1) * S)

            w1_sb = rpool.tile([P, KO * GW], bf, tag="w1")
            nc.sync.dma_start(w1_sb[:, : 4 * GW], w1[:, : 4 * GW])
            nc.scalar.dma_start(w1_sb[:, 4 * GW :], w1[:, 4 * GW :])

            bias_sb = rpool.tile([P, OT], f32, tag="bias")
            nc.gpsimd.dma_start(bias_sb[:], bias[:, :])

            def wsl(k, ot):
                if ot < 4:
                    base = k * S + T_pad + ot * P
                    return xw_sb[:, base : base + P]
                base = k * GW + (ot - 4) * P
                return w1_sb[:, base : base + P]

            def xsl(k, lo, hi):
                return xw_sb[:, k * S + lo : k * S + hi]

            # ---- PE warm-up -------------------------------------------------
            # HAM un-throttles after ~3.4us of sustained PE activity; dummy
            # matmuls during the initial DMA wait start that clock early.
            # Results land in psA0 which the first real MM (start=True)
            # overwrites.
            warm_sb = rpool.tile([P, P], bf, tag="warm")
            nc.vector.memset(warm_sb[:], 0.0)

            def emit_out(ot, psA, psB):
                o_sb = opool.tile([P, T_pad], bf, tag="ot", name=f"o{ot}")
                bcol = bias_sb[:, ot : ot + 1]
                nc.vector.tensor_scalar_add(o_sb[:, :TA], psA[:], bcol)
                if TB:
                    nc.scalar.activation(
                        o_sb[:, TA:],
                        psB[:],
                        mybir.ActivationFunctionType.Identity,
                        bias=bcol,
                        scale=1.0,
                    )
                eng = nc.sync if ot % 2 == 0 else nc.scalar
                eng.dma_start(out[ot * P : (ot + 1) * P, :], o_sb[:])

            def mk_psum(i, nm):
                psA = psum_pool.tile([P, TA], f32, tag=f"psA{i}", name=f"psA{nm}")
                psB = (
                    psum_pool.tile([P, TB], f32, tag=f"psB{i}", name=f"psB{nm}")
                    if TB
                    else None
                )
                return psA, psB

            # ---- phase 0: o-tiles 0-3, k-outer (DMA-paced) ------------------
            ps = [mk_psum(i, f"p0_{i}") for i in range(4)]

            for _ in range(N_WARM):
                nc.tensor.matmul(
                    ps[0][0][:, :P],
                    lhsT=warm_sb[:],
                    rhs=warm_sb[:],
                    start=True,
                    stop=True,
                )

            for k in range(KO):
                for i in range(4):
                    lhsT = wsl(k, i)
                    nc.tensor.matmul(
                        ps[i][0][:],
                        lhsT=lhsT,
                        rhs=xsl(k, 0, TA),
                        start=(k == 0),
                        stop=(k == KO - 1),
                    )
                    if TB:
                        nc.tensor.matmul(
                            ps[i][1][:],
                            lhsT=lhsT,
                            rhs=xsl(k, TA, T_pad),
                            start=(k == 0),
                            stop=(k == KO - 1),
                        )
            for i in range(4):
                emit_out(i, ps[i][0], ps[i][1])

            # ---- phase 1: o-tiles 4-7, ot-outer (SBUF-resident) -------------
            # Per-half psum groups: the A-half's bias-add runs on DVE while
            # the B-half's matmuls stream, so only the final half's add +
            # store is kernel tail. The last o-tile's store is split across
            # both HWDGE queues to halve its wire+issue time.
            for i in range(4):
                ot = 4 + i
                psA, psB = mk_psum(i, f"p1_{i}")
                o_sb = opool.tile([P, T_pad], bf, tag="ot", name=f"o{ot}")
                bcol = bias_sb[:, ot : ot + 1]
                last = ot == OT - 1
                orow = out[ot * P : (ot + 1) * P, :]
                for k in range(KO):
                    nc.tensor.matmul(
                        psA[:],
                        lhsT=wsl(k, ot),
                        rhs=xsl(k, 0, TA),
                        start=(k == 0),
                        stop=(k == KO - 1),
                    )
                nc.vector.tensor_scalar_add(o_sb[:, :TA], psA[:], bcol)
                if last and TB:
                    # critical tail: A half stores while the B half's
                    # matmuls stream; B's bias-add is split DVE/ACT and
                    # its store rides the other (idle) HWDGE ring
                    nc.sync.dma_start(orow[:, :TA], o_sb[:, :TA])
                if TB:
                    for k in range(KO):
                        nc.tensor.matmul(
                            psB[:],
                            lhsT=wsl(k, ot),
                            rhs=xsl(k, TA, T_pad),
                            start=(k == 0),
                            stop=(k == KO - 1),
                        )
                    nc.scalar.activation(
                        o_sb[:, TA:],
                        psB[:],
                        mybir.ActivationFunctionType.Identity,
                        bias=bcol,
                        scale=1.0,
                    )
                    if last:
                        nc.scalar.dma_start(orow[:, TA:], o_sb[:, TA:])
                if not last:
                    eng = nc.sync if ot % 2 == 0 else nc.scalar
                    eng.dma_start(orow, o_sb[:])
                elif not TB:
                    nc.sync.dma_start(orow, o_sb[:])
    nc.finalize()
    return nc


def kernel(x, category_id, weight, bias):
    global LAST_RESULTS
    x = np.asarray(x)
    category_id = np.asarray(category_id)
    weight = np.ascontiguousarray(np.asarray(weight), dtype=np.float32)
    bias = np.ascontiguousarray(np.asarray(bias), dtype=np.float32)

    orig_shape = x.shape
    D = orig_shape[-1]
    C, _, O = weight.shape
    KO, OT = D // P, O // P
    assert C == N_CORES and KO == 8 and OT == 8

    T = int(np.prod(orig_shape[:-1]))
    x_flat = np.ascontiguousarray(x.reshape(T, D), dtype=np.float32)
    cid = category_id.reshape(T).astype(np.int64)

    idx_per_c = [np.flatnonzero(cid == c) for c in range(C)]
    counts = [len(ix) for ix in idx_per_c]
    T_pad = max(32, -(-max(counts) // 16) * 16)

    key = (T_pad, D, O)
    if key not in _nc_cache:
        _nc_cache[key] = _build_nc(T_pad, D, O)
    nc = _nc_cache[key]

    # pre-arranged per-partition-contiguous layouts (see _build_nc)
    in_maps = []
    for c in range(C):
        xc = np.zeros((T_pad, D), dtype=np.float32)
        xc[: counts[c]] = x_flat[idx_per_c[c]]
        # [t, (k p)] -> [p, k, t]
        xh = xc.T.reshape(KO, P, T_pad).transpose(1, 0, 2).astype(BF16)
        # [(k p), (g i o)] -> [g, p, k, i*o]   (g = ot//4, i = ot%4)
        wh = (
            weight[c]
            .reshape(KO, P, 2, 4 * P)
            .transpose(2, 1, 0, 3)
            .astype(BF16)
        )
        # combined per-k stream for phase 0: [p][k][x_t | w_g0]
        xwh = np.ascontiguousarray(
            np.concatenate([xh, wh[0]], axis=2)
        ).reshape(P, KO * (T_pad + 4 * P))
        w1h = np.ascontiguousarray(wh[1]).reshape(P, KO * 4 * P)
        # [(ot o)] -> [o, ot]
        bh = np.ascontiguousarray(bias[c].reshape(OT, P).T)
        in_maps.append({"xw": xwh, "w1": w1h, "bias": bh})

    res = run_bass_kernel_spmd(nc, in_maps, list(range(N_CORES)))
    LAST_RESULTS = res

    out_flat = np.empty((T, O), dtype=np.float32)
    for c in range(C):
        oc = np.asarray(res.results[c]["out"])  # [O, T_pad] bf16
        out_flat[idx_per_c[c]] = oc[:, : counts[c]].T.astype(np.float32)
    return out_flat.reshape(*orig_shape[:-1], O)


# revision 27
# speedup vs baseline: 1.4802x; 1.0198x over previous
"""CategorySpecificLinear Trainium2 kernel (v3: bf16 weight-stationary).

out[t] = x[t] @ weight[category_id[t]] + bias[category_id[t]]

Strategy: expert-parallel over the 8 categories (C == n_cores == 8).
Host routes tokens by category; core c computes its category's tokens.

Device-side formulation (per core, transposed output):
    outT[o, t] = sum_k wT[k, o] * xT[k, t] + bias[o]
with the weight tile [128k x 128o] STATIONARY in the PE array and x
streamed as the moving operand, so PE stream cycles = OT*KO*T_pad
(~34k cycles ~ 14.1 us warm @2.4GHz) with zero m-tile quantization
waste. All operands bf16 (psum accumulates fp32) -> half the HBM
traffic of fp32 and 1 col/cycle warm on the PE.

Schedule (8 psum banks = 4 o-tiles x 2 T-halves in flight):
  phase 0 (o-tiles 0-3): k-OUTER -- each k-step needs only x[k] and
    w[g0,k], so the PE starts ~1.5 us after the first small chunks
    land instead of after the full 3 MB input load.
  phase 1 (o-tiles 4-7): everything is SBUF-resident by now, so run
    ot-OUTER: each o-tile's output drains (DVE/ACT bias-add + bf16
    store) while the next o-tile computes -> only the last o-tile's
    add+store is kernel tail.
Warm-up: ~24 dummy matmuls bridge the initial DMA wait so the HAM
clock gate reaches 8/8 before the real stream.

DMA: three queues (sync + scalar HWDGE, gpsimd SWDGE) each see
~190 GB/s when all active and lose ~1 us between chained DMAs, so
inputs are split in 2 chunks per queue (small first chunk for early
PE start): sync = x, scalar = w[o-tiles 0-3], gpsimd = bias + w[o-
tiles 4-7]. Host pre-arranges x as [p][k][t] and w as [p][g][k][o]
(per-partition contiguous) so every load is a plain 2D slice.
Per-core HBM: x 1.06 + w 2 + out 1.06 MB = 4.2 MB.

bf16 numerics: rel err ~3e-3 on dot-1024 (gate is 2e-2).
"""

import contextlib
import ctypes
import os
import sys
import types

import numpy as np

sys.path.insert(0, "/opt/trn_rl_repo")


def _ensure_ntff_hook():
    """Provide antenv.axon_hooks if the image lacks it.

    concourse.bass_utils imports antenv.axon_hooks.get_axon_ntff_profile_hook
    when trace=True under axon; some agent images don't ship that module, in
    which case the boot's NTFF hook registration silently degrades and the
    import in bass_utils crashes. Recreate the slim ctypes hook here
    (mirrors trn_agent_boot.trn_boot._ntff_profile_via_ctypes).
    """
    try:
        import antenv.axon_hooks  # noqa: F401

        return
    except ImportError:
        pass

    so_path = "/opt/axon/libaxon_pjrt.so"
    hook = None
    if os.path.exists(so_path):
        lib = ctypes.CDLL(so_path)
        if hasattr(lib, "axon_start_nrt_profile"):
            lib.axon_start_nrt_profile.argtypes = [
                ctypes.POINTER(ctypes.c_int64),
                ctypes.c_size_t,
            ]
            lib.axon_start_nrt_profile.restype = ctypes.c_int64
            lib.axon_stop_nrt_profile.argtypes = [ctypes.c_char_p]
            lib.axon_stop_nrt_profile.restype = ctypes.c_int64

            @contextlib.contextmanager
            def hook(output_dir, device_ids):
                import jax

                jax.devices()
                if device_ids:
                    ids = (ctypes.c_int64 * len(device_ids))(*device_ids)
                    rc = lib.axon_start_nrt_profile(ids, len(device_ids))
                else:
                    rc = lib.axon_start_nrt_profile(None, 0)
                if rc != 0:
                    raise RuntimeError(f"axon_start_nrt_profile rc={rc}")
                try:
                    yield
                finally:
                    n = lib.axon_stop_nrt_profile(str(output_dir).encode())
                    if n <= 0:
                        print(
                            f"ntff profile: rc={n} writing {output_dir}",
                            file=sys.stderr,
                        )

    mod = types.ModuleType("antenv.axon_hooks")
    _state = {"hook": hook}
    mod.set_axon_ntff_profile_hook = lambda h: _state.__setitem__("hook", h)
    mod.get_axon_ntff_profile_hook = lambda: _state["hook"]
    sys.modules["antenv.axon_hooks"] = mod
    try:
        import antenv

        antenv.axon_hooks = mod
    except ImportError:
        pass


_ensure_ntff_hook()

import ml_dtypes

import concourse.bass as bass
import concourse.bacc as bacc_mod
import concourse.mybir as mybir
import concourse.tile as tile
from concourse.bass_utils import run_bass_kernel_spmd

N_CORES = 8
P = 128
BF16 = np.dtype(ml_dtypes.bfloat16)

_nc_cache = {}
LAST_RESULTS = None  # BassKernelResults of the most recent run (for test.py)

N_WARM = 30  # dummy matmuls bridging the initial DMA wait (HAM warm-up)


def _build_nc(T_pad: int, D: int, O: int):
    KO = D // P
    OT = O // P
    assert KO == 8 and OT == 8
    bf = mybir.dt.bfloat16
    f32 = mybir.dt.float32

    # moving-operand split: one psum bank holds <=512 fp32 per partition,
    # so stream T in two pieces. Asymmetric (big A, 128-wide B) so the
    # kernel tail -- the last o-tile's B bias-add + store -- is small;
    # total stream cycles are unchanged.
    if T_pad <= 512:
        TA, TB = T_pad, 0
    elif T_pad <= 512 + 128:
        TA, TB = T_pad - 128, 128
    else:
        TA = -(-(T_pad // 2) // 16) * 16
        TB = T_pad - TA
        assert TB <= 512

    S = T_pad + 4 * P  # per-k elems per partition in xw (x slice + 4 o-tiles of w)
    GW = 4 * P  # per-k elems per partition in w1 (512)

    nc = bacc_mod.Bacc()
    xw = nc.dram_tensor("xw", [P, KO * S], bf, kind="ExternalInput")
    w1 = nc.dram_tensor("w1", [P, KO * GW], bf, kind="ExternalInput")
    bias = nc.dram_tensor("bias", [P, OT], f32, kind="ExternalInput")
    out = nc.dram_tensor("out", [O, T_pad], bf, kind="ExternalOutput")

    with tile.TileContext(nc) as tc:
        with (
            tc.tile_pool(name="resident", bufs=1) as rpool,
            tc.tile_pool(name="psum", bufs=1, space="PSUM") as psum_pool,
            tc.tile_pool(name="obuf", bufs=8) as opool,
        ):
            # ---- input DMAs -------------------------------------------------
            # The DMA rings round-robin fairly at packet granularity, so
            # ring FIFO order is the only priority mechanism. Phase 0's
            # data is ONE combined per-k stream (x slice + its 4 weight
            # tiles, exactly the PE's per-k consumption) alternated
            # across both HWDGE rings; k0/k1 are split x-vs-w across the
            # rings so the pipeline fills fast. The phase-1 weights ride
            # the same rings' tails in k order, so they never steal
            # bandwidth from the k-paced phase-0 stream. gpsimd carries
            # bias + the non-critical output stores.
            # memset first so it's the gpsimd engine's first instruction:
            # the dummy-matmul warm-up stream can then start ~1us earlier
            warm_sb = rpool.tile([P, P], bf, tag="warm")
            nc.gpsimd.memset(warm_sb[:], 0.0)

            xw_sb = rpool.tile([P, KO * S], bf, tag="xw")

            def ldxw(eng, lo, hi):
                eng.dma_start(xw_sb[:, lo:hi], xw[:, lo:hi])

            ldxw(nc.sync, 0, T_pad)  # x k0
            ldxw(nc.scalar, T_pad, S)  # w k0
            ldxw(nc.gpsimd, S, 2 * S)  # k1 block rides the third (SWDGE) path
            for k in range(2, KO):
                ldxw(nc.sync if k % 2 == 0 else nc.scalar, k * S, (k + 1) * S)

            w1_sb = rpool.tile([P, KO * GW], bf, tag="w1")
            nc.sync.dma_start(w1_sb[:, : 4 * GW], w1[:, : 4 * GW])
            nc.scalar.dma_start(w1_sb[:, 4 * GW :], w1[:, 4 * GW :])

            bias_sb = rpool.tile([P, OT], f32, tag="bias")
            nc.gpsimd.dma_start(bias_sb[:], bias[:, :])

            def wsl(k, ot):
                if ot < 4:
                    base = k * S + T_pad + ot * P
                    return xw_sb[:, base : base + P]
                base = k * GW + (ot - 4) * P
                return w1_sb[:, base : base + P]

            def xsl(k, lo, hi):
                return xw_sb[:, k * S + lo : k * S + hi]

            def emit_out(ot, psA, psB):
                o_sb = opool.tile([P, T_pad], bf, tag="ot", name=f"o{ot}")
                bcol = bias_sb[:, ot : ot + 1]
                nc.vector.tensor_scalar_add(o_sb[:, :TA], psA[:], bcol)
                if TB:
                    nc.scalar.activation(
                        o_sb[:, TA:],
                        psB[:],
                        mybir.ActivationFunctionType.Identity,
                        bias=bcol,
                        scale=1.0,
                    )
                eng = nc.sync if ot % 2 == 0 else nc.scalar
                eng.dma_start(out[ot * P : (ot + 1) * P, :], o_sb[:])

            def mk_psum(i, nm):
                psA = psum_pool.tile([P, TA], f32, tag=f"psA{i}", name=f"psA{nm}")
                psB = (
                    psum_pool.tile([P, TB], f32, tag=f"psB{i}", name=f"psB{nm}")
                    if TB
                    else None
                )
                return psA, psB

            # ---- phase 0: o-tiles 0-3, k-outer (DMA-paced) ------------------
            ps = [mk_psum(i, f"p0_{i}") for i in range(4)]

            for _ in range(N_WARM):
                nc.tensor.matmul(
                    ps[0][0][:, :P],
                    lhsT=warm_sb[:],
                    rhs=warm_sb[:],
                    start=True,
                    stop=True,
                )

            for k in range(KO):
                for i in range(4):
                    lhsT = wsl(k, i)
                    nc.tensor.matmul(
                        ps[i][0][:],
                        lhsT=lhsT,
                        rhs=xsl(k, 0, TA),
                        start=(k == 0),
                        stop=(k == KO - 1),
                    )
                    if TB:
                        nc.tensor.matmul(
                            ps[i][1][:],
                            lhsT=lhsT,
                            rhs=xsl(k, TA, T_pad),
                            start=(k == 0),
                            stop=(k == KO - 1),
                        )
            for i in range(4):
                emit_out(i, ps[i][0], ps[i][1])

            # ---- phase 1: o-tiles 4-7, ot-outer (SBUF-resident) -------------
            # Per-half psum groups: the A-half's bias-add runs on DVE while
            # the B-half's matmuls stream, so only the final half's add +
            # store is kernel tail. The last o-tile's store is split across
            # both HWDGE queues to halve its wire+issue time.
            for i in range(4):
                ot = 4 + i
                psA, psB = mk_psum(i, f"p1_{i}")
                o_sb = opool.tile([P, T_pad], bf, tag="ot", name=f"o{ot}")
                bcol = bias_sb[:, ot : ot + 1]
                last = ot == OT - 1
                orow = out[ot * P : (ot + 1) * P, :]
                for k in range(KO):
                    nc.tensor.matmul(
                        psA[:],
                        lhsT=wsl(k, ot),
                        rhs=xsl(k, 0, TA),
                        start=(k == 0),
                        stop=(k == KO - 1),
                    )
                nc.vector.tensor_scalar_add(o_sb[:, :TA], psA[:], bcol)
                if last and TB:
                    # critical tail: A half stores while the B half's
                    # matmuls stream; B's bias-add is split DVE/ACT and
                    # its store rides the other (idle) HWDGE ring
                    nc.sync.dma_start(orow[:, :TA], o_sb[:, :TA])
                if TB:
                    for k in range(KO):
                        nc.tensor.matmul(
                            psB[:],
                            lhsT=wsl(k, ot),
                            rhs=xsl(k, TA, T_pad),
                            start=(k == 0),
                            stop=(k == KO - 1),
                        )
                    nc.scalar.activation(
                        o_sb[:, TA:],
                        psB[:],
                        mybir.ActivationFunctionType.Identity,
                        bias=bcol,
                        scale=1.0,
                    )
                    if last:
                        nc.scalar.dma_start(orow[:, TA:], o_sb[:, TA:])
                if not last:
                    eng = nc.sync if ot % 2 == 0 else nc.scalar
                    eng.dma_start(orow, o_sb[:])
                elif not TB:
                    nc.sync.dma_start(orow, o_sb[:])
    nc.finalize()
    return nc


def kernel(x, category_id, weight, bias):
    global LAST_RESULTS
    x = np.asarray(x)
    category_id = np.asarray(category_id)
    weight = np.ascontiguousarray(np.asarray(weight), dtype=np.float32)
    bias = np.ascontiguousarray(np.asarray(bias), dtype=np.float32)

    orig_shape = x.shape
    D = orig_shape[-1]
    C, _, O = weight.shape
    KO, OT = D // P, O // P
    assert C == N_CORES and KO == 8 and OT == 8

    T = int(np.prod(orig_shape[:-1]))
    x_flat = np.ascontiguousarray(x.reshape(T, D), dtype=np.float32)
    cid = category_id.reshape(T).astype(np.int64)

    idx_per_c = [np.flatnonzero(cid == c) for c in range(C)]
    counts = [len(ix) for ix in idx_per_c]
    T_pad = max(32, -(-max(counts) // 16) * 16)

    key = (T_pad, D, O)
    if key not in _nc_cache:
        _nc_cache[key] = _build_nc(T_pad, D, O)
    nc = _nc_cache[key]

    # pre-arranged per-partition-contiguous layouts (see _build_nc)
    in_maps = []
    for c in range(C):
        xc = np.zeros((T_pad, D), dtype=np.float32)
        xc[: counts[c]] = x_flat[idx_per_c[c]]
        # [t, (k p)] -> [p, k, t]
        xh = xc.T.reshape(KO, P, T_pad).transpose(1, 0, 2).astype(BF16)
        # [(k p), (g i o)] -> [g, p, k, i*o]   (g = ot//4, i = ot%4)
        wh = (
            weight[c]
            .reshape(KO, P, 2, 4 * P)
            .transpose(2, 1, 0, 3)
            .astype(BF16)
        )
        # combined per-k stream for phase 0: [p][k][x_t | w_g0]
        xwh = np.ascontiguousarray(
            np.concatenate([xh, wh[0]], axis=2)
        ).reshape(P, KO * (T_pad + 4 * P))
        w1h = np.ascontiguousarray(wh[1]).reshape(P, KO * 4 * P)
        # [(ot o)] -> [o, ot]
        bh = np.ascontiguousarray(bias[c].reshape(OT, P).T)
        in_maps.append({"xw": xwh, "w1": w1h, "bias": bh})

    res = run_bass_kernel_spmd(nc, in_maps, list(range(N_CORES)))
    LAST_RESULTS = res

    out_flat = np.empty((T, O), dtype=np.float32)
    for c in range(C):
        oc = np.asarray(res.results[c]["out"])  # [O, T_pad] bf16
        out_flat[idx_per_c[c]] = oc[:, : counts[c]].T.astype(np.float32)
    return out_flat.reshape(*orig_shape[:-1], O)
